# revision 7
# baseline (speedup 1.0000x reference)
"""Trainium2 Bass kernel: pre-LN single-head causal attention + residual.

Reference computation (B=4, S=2048, H=K=2048, fp32):
    xn = LayerNorm(x) * gamma + beta
    q,k,v = xn @ qkv (split)
    out = causal_softmax(q k^T / sqrt(K)) @ v @ o_proj + x

Sharding: 8 cores = 4 batches x 2 query-halves. Each core gets its batch's
rows PERMUTED so that its query rows sit at fixed positions, arranged in 4
"classes" of 256 query rows whose causal key extent is 512*(e+1) rows --
a load-balanced folded-causal split with identical program shape on all
cores (pure SPMD; per-core behavior comes only from input data: the
permuted x and the causality masks).

On-device pipeline per core (all matmuls bf16, fp32 accumulate in PSUM):
  A0: LayerNorm stats (bn_stats) -> (x-mu)*rstd on ACT -> PE-transpose ->
      evacuate with gamma/beta fold -> x_norm^T bf16 [hid_p, tok_f]
  A1: v and k^T (spilled to DRAM), q^T resident (bf16)
  B:  scores^T = k^T-tiles^T @ q^T per class, exp on ACT (no max-subtract:
      |score*scale| < ~4 so fp32 exp is exact), causal mask multiply,
      denominators via ones-matmul
  C:  out^T = v-tiles^T @ attn^T (v streamed back from DRAM)
  D:  y = diag(1/sums) (out @ o_proj) + x  (normalization folded into the
      PSUM eviction as a per-partition scale; residual added in same op)
"""
import sys

import numpy as np

sys.path.insert(0, "/opt/trn_rl_repo")


def _install_ntff_hook():
    """Register the axon NTFF profile hook bass_utils expects (the image's
    antenv package lacks axon_hooks); degrades to no-op when unavailable."""
    import types
    if "antenv.axon_hooks" in sys.modules:
        return
    try:
        from trn_agent_boot.trn_boot import _ntff_profile_via_ctypes
        hook = _ntff_profile_via_ctypes("/opt/axon/libaxon_pjrt.so")
    except Exception:
        hook = None
    m = types.ModuleType("antenv.axon_hooks")
    m.get_axon_ntff_profile_hook = lambda: hook
    sys.modules["antenv.axon_hooks"] = m


_install_ntff_hook()

import ml_dtypes  # noqa: E402
import concourse.bass as bass  # noqa: E402
import concourse.tile as tile  # noqa: E402
from concourse import bacc, mybir  # noqa: E402
from concourse.bass_utils import run_bass_kernel_spmd  # noqa: E402

F32 = mybir.dt.float32
BF16 = mybir.dt.bfloat16
AF = mybir.ActivationFunctionType
OP = mybir.AluOpType

B, S, H, KEY = 4, 2048, 2048, 2048
NCHK = 16                 # 128-row chunks per sequence
EPS = 1e-5
SCALE = 1.0 / float(np.sqrt(KEY))
ABASE = [0, 4, 12, 24]    # attn^T tile base index per class
ATOT = 40                 # total k-chunk tiles across classes
PC = [0, 1, 4, 5, 8, 9, 12, 13]   # position chunks holding this core's q rows


def perm_chunks(h):
    out = []
    for e in range(4):
        out += [4 * e + 2 * h, 4 * e + 2 * h + 1,
                4 * e + 2 * (1 - h), 4 * e + 2 * (1 - h) + 1]
    return out


def build():
    nc = bacc.Bacc("TRN2", target_bir_lowering=False, debug=False)

    x_d = nc.dram_tensor("x", [S, H], F32, kind="ExternalInput")
    wqkv_d = nc.dram_tensor("wqkv", [H, 3 * KEY], BF16, kind="ExternalInput")
    wo_d = nc.dram_tensor("wo", [KEY, H], BF16, kind="ExternalInput")
    gamma_d = nc.dram_tensor("gamma", [H], F32, kind="ExternalInput")
    beta_d = nc.dram_tensor("beta", [H], F32, kind="ExternalInput")
    mask_d = nc.dram_tensor("mask", [4, 4, 128, 256], BF16, kind="ExternalInput")
    y_d = nc.dram_tensor("y", [1024, H], F32, kind="ExternalOutput")
    vsp_d = nc.dram_tensor("vsp", [NCHK, 128, KEY], BF16, kind="Internal")
    ssp_d = nc.dram_tensor("ssp", [4, 256], F32, kind="Internal")
    ksp_d = nc.dram_tensor("ksp", [NCHK, 128, S], BF16, kind="Internal")

    ident = nc.inline_tensor(np.eye(128, dtype=np.float32), name="ident")

    with tile.TileContext(nc) as tc:
        with (
            tc.tile_pool(name="small", bufs=1) as small,
            tc.tile_pool(name="p_main", bufs=1) as p_main,
        ):
            xnT = p_main.tile([128, NCHK, S], BF16)   # x_norm^T  [hid, tok]
            qT = p_main.tile([128, NCHK, 1024], BF16)  # q^T [key, class-packed q]
            recip = small.tile([128, 8], F32)         # 1/sums per q-chunk
            gcol = small.tile([128, NCHK], F32)       # gamma, [p, hc]
            bcol = small.tile([128, NCHK], F32)       # beta
            ones = small.tile([128, 1], BF16)
            id_sb = small.tile([128, 128], F32)

            nc.sync.dma_start(gcol[:], gamma_d[:].rearrange("(c p) -> p c", p=128))
            nc.sync.dma_start(bcol[:], beta_d[:].rearrange("(c p) -> p c", p=128))
            nc.sync.dma_start(id_sb[:], ident[:])
            nc.vector.memset(ones[:], 1.0)

            # ---------- A0: LN + transpose ----------
            with (
                nc.named_scope("ln_transpose"),
                tc.tile_pool(name="a0x", bufs=2) as a0x,
                tc.tile_pool(name="a0xp", bufs=5) as a0xp,
                tc.tile_pool(name="a0s", bufs=4) as a0s,
                tc.tile_pool(name="pp_tr", bufs=2, space=bass.MemorySpace.PSUM) as pp_tr,
            ):
                for tg in range(4):
                    xps = []
                    for i in range(4):
                        tci = tg * 4 + i
                        x_t = a0x.tile([128, H], F32, tag="x")
                        nc.sync.dma_start(x_t[:], x_d[tci * 128:(tci + 1) * 128, :])
                        st = a0s.tile([128, 4, 6], F32, tag="st")
                        for j in range(4):
                            nc.vector.bn_stats(st[:, j, :], x_t[:, j * 512:(j + 1) * 512])
                        ag = a0s.tile([128, 2], F32, tag="ag")
                        nc.vector.bn_aggr(ag[:], st[:])
                        veps = a0s.tile([128, 1], F32, tag="veps")
                        nc.vector.tensor_scalar_add(veps[:], ag[:, 1:2], EPS)
                        sq = a0s.tile([128, 1], F32, tag="sq")
                        nc.scalar.sqrt(sq[:], veps[:])
                        rstd = a0s.tile([128, 1], F32, tag="rstd")
                        nc.vector.reciprocal(rstd[:], sq[:])
                        nmr = a0s.tile([128, 1], F32, tag="nmr")
                        nc.vector.tensor_scalar(nmr[:], ag[:, 0:1], rstd[:], -1.0,
                                                OP.mult, OP.mult)
                        xp = a0xp.tile([128, H], F32, tag="xp")
                        nc.scalar.activation(xp[:], x_t[:], AF.Identity,
                                             bias=nmr[:], scale=rstd[:])
                        xps.append(xp)
                    for hc in range(NCHK):
                        ps = pp_tr.tile([128, 512], F32, tag="tr")
                        for i in range(4):
                            nc.tensor.transpose(ps[:, i * 128:(i + 1) * 128],
                                                xps[i][:, hc * 128:(hc + 1) * 128],
                                                id_sb[:])
                        dst = xnT[:, hc, tg * 512:(tg + 1) * 512]
                        if hc % 2 == 0:
                            nc.vector.tensor_scalar(dst, ps[:], gcol[:, hc:hc + 1],
                                                    bcol[:, hc:hc + 1], OP.mult, OP.add)
                        else:
                            nc.scalar.activation(dst, ps[:], AF.Identity,
                                                 bias=bcol[:, hc:hc + 1],
                                                 scale=gcol[:, hc:hc + 1])

            # ---------- A1a: v = xn @ Wv -> DRAM spill ----------
            with (
                nc.named_scope("v_proj"),
                tc.tile_pool(name="wv", bufs=1) as wvp,
                tc.tile_pool(name="vst", bufs=4) as vst,
                tc.tile_pool(name="pp_v", bufs=4, space=bass.MemorySpace.PSUM) as pp_v,
            ):
                wv_sb = wvp.tile([128, NCHK, KEY], BF16)
                for hc in range(NCHK):
                    nc.sync.dma_start(wv_sb[:, hc, :],
                                      wqkv_d[hc * 128:(hc + 1) * 128, 2 * KEY:3 * KEY])
                for tci in range(NCHK):
                    for kvt in range(4):
                        ps = pp_v.tile([128, 512], F32, tag="v")
                        for hc in range(NCHK):
                            nc.tensor.matmul(ps[:],
                                             xnT[:, hc, tci * 128:(tci + 1) * 128],
                                             wv_sb[:, hc, kvt * 512:(kvt + 1) * 512],
                                             start=(hc == 0), stop=(hc == NCHK - 1))
                        vs = vst.tile([128, 512], BF16, tag="vs")
                        nc.vector.tensor_copy(vs[:], ps[:])
                        nc.sync.dma_start(vsp_d[tci][:, kvt * 512:(kvt + 1) * 512], vs[:])

            # ---------- A1b: q^T (resident), k^T (spilled) ----------
            with (
                nc.named_scope("qk_proj"),
                tc.tile_pool(name="wqk", bufs=8) as wqk,
                tc.tile_pool(name="kst", bufs=6) as kst,
                tc.tile_pool(name="pp_q", bufs=3, space=bass.MemorySpace.PSUM) as pp_q,
                tc.tile_pool(name="pp_k", bufs=5, space=bass.MemorySpace.PSUM) as pp_k,
            ):
                for kc in range(NCHK):
                    psq_a = pp_q.tile([128, 512], F32, tag="q")
                    psq_b = pp_q.tile([128, 512], F32, tag="q")
                    psk = [pp_k.tile([128, 512], F32, tag="k", name=f"psk{t4}")
                           for t4 in range(4)]
                    for hc in range(NCHK):
                        wq_t = wqk.tile([128, 128], BF16, tag="wq")
                        nc.sync.dma_start(
                            wq_t[:], wqkv_d[hc * 128:(hc + 1) * 128,
                                            kc * 128:(kc + 1) * 128])
                        wk_t = wqk.tile([128, 128], BF16, tag="wk")
                        nc.sync.dma_start(
                            wk_t[:], wqkv_d[hc * 128:(hc + 1) * 128,
                                            KEY + kc * 128:KEY + (kc + 1) * 128])
                        xr3 = xnT[:, hc, :].rearrange("p (g r) -> p g r", r=256)
                        st, sp = (hc == 0), (hc == NCHK - 1)
                        nc.tensor.matmul(psq_a[:], wq_t[:], xr3[:, 0:3:2, :],
                                         start=st, stop=sp)
                        nc.tensor.matmul(psq_b[:], wq_t[:], xr3[:, 4:7:2, :],
                                         start=st, stop=sp)
                        for t4 in range(4):
                            nc.tensor.matmul(psk[t4][:], wk_t[:],
                                             xnT[:, hc, t4 * 512:(t4 + 1) * 512],
                                             start=st, stop=sp)
                    nc.vector.tensor_copy(qT[:, kc, 0:512], psq_a[:])
                    nc.vector.tensor_copy(qT[:, kc, 512:1024], psq_b[:])
                    for t4 in range(4):
                        ks = kst.tile([128, 512], BF16, tag="ks")
                        if t4 % 2 == 0:
                            nc.scalar.copy(ks[:], psk[t4][:])
                        else:
                            nc.vector.tensor_copy(ks[:], psk[t4][:])
                        nc.sync.dma_start(ksp_d[kc][:, t4 * 512:(t4 + 1) * 512], ks[:])

            # ---------- B..D pools: attn^T and out^T ----------
            with tc.tile_pool(name="p_bc", bufs=1) as p_bc:
                aT = p_bc.tile([128, ATOT, 256], BF16)     # attn^T tiles
                oT = p_bc.tile([128, NCHK, 1024], BF16)    # out^T [kv, q]

                # ---------- B: scores^T -> exp -> mask -> attn^T, denominators ----
                with (
                    nc.named_scope("scores"),
                    tc.tile_pool(name="bk", bufs=24) as bk,
                    tc.tile_pool(name="bm", bufs=4) as bm,
                    tc.tile_pool(name="bs", bufs=4) as bs,
                    tc.tile_pool(name="pp_s", bufs=3, space=bass.MemorySpace.PSUM) as pp_s,
                    tc.tile_pool(name="pp_sum", bufs=2,
                                 space=bass.MemorySpace.PSUM) as pp_sum,
                ):
                    for e in range(4):
                        kext = 4 * (e + 1)
                        ps_sum = pp_sum.tile([1, 256], F32, tag="sum")
                        for kch in range(kext):
                            ps_s = pp_s.tile([128, 256], F32, tag="s")
                            for kc in range(NCHK):
                                kt = bk.tile([128, 128], BF16, tag="kt")
                                nc.sync.dma_start(
                                    kt[:], ksp_d[kc][:, kch * 128:(kch + 1) * 128])
                                nc.tensor.matmul(ps_s[:], kt[:],
                                                 qT[:, kc, e * 256:(e + 1) * 256],
                                                 start=(kc == 0), stop=(kc == NCHK - 1))
                            dst = aT[:, ABASE[e] + kch, :]
                            if kch >= 4 * e:
                                tmp = bs.tile([128, 256], BF16, tag="exps")
                                nc.scalar.activation(tmp[:], ps_s[:], AF.Exp, scale=SCALE)
                                mt = bm.tile([128, 256], BF16, tag="mask")
                                nc.sync.dma_start(mt[:], mask_d[e, kch - 4 * e])
                                nc.vector.tensor_mul(dst, tmp[:], mt[:])
                            else:
                                nc.scalar.activation(dst, ps_s[:], AF.Exp, scale=SCALE)
                            nc.tensor.matmul(ps_sum[:], ones[:, 0:1], dst,
                                             start=(kch == 0), stop=(kch == kext - 1))
                        srow = bs.tile([1, 256], F32, tag="srow")
                        nc.vector.tensor_copy(srow[:], ps_sum[:])
                        nc.sync.dma_start(ssp_d[e], srow[:])
                        scol = bs.tile([128, 2], F32, tag="scol")
                        nc.sync.dma_start(scol[:],
                                          ssp_d[e].rearrange("(j p) -> p j", p=128))
                        nc.vector.reciprocal(recip[:, 2 * e:2 * e + 2], scol[:])

                # ---------- C: out^T = v^T-chunks @ attn^T ----------
                with (
                    nc.named_scope("attn_v"),
                    tc.tile_pool(name="cv", bufs=16) as cv,
                    tc.tile_pool(name="pp_o", bufs=4, space=bass.MemorySpace.PSUM) as pp_o,
                ):
                    for e in range(4):
                        kext = 4 * (e + 1)
                        for kvc in range(NCHK):
                            ps_o = pp_o.tile([128, 256], F32, tag="o")
                            for kch in range(kext):
                                vt = cv.tile([128, 128], BF16, tag="vt")
                                nc.sync.dma_start(
                                    vt[:], vsp_d[kch][:, kvc * 128:(kvc + 1) * 128])
                                nc.tensor.matmul(ps_o[:], vt[:],
                                                 aT[:, ABASE[e] + kch, :],
                                                 start=(kch == 0), stop=(kch == kext - 1))
                            if kvc % 2 == 0:
                                nc.scalar.copy(oT[:, kvc, e * 256:(e + 1) * 256],
                                               ps_o[:])
                            else:
                                nc.vector.tensor_copy(oT[:, kvc, e * 256:(e + 1) * 256],
                                                      ps_o[:])

                # ---------- D: y = diag(1/sums) (out @ Wo) + x ----------
                with (
                    nc.named_scope("o_proj"),
                    tc.tile_pool(name="dw", bufs=4) as dw,
                    tc.tile_pool(name="dx", bufs=4) as dx,
                    tc.tile_pool(name="dy", bufs=4) as dy,
                    tc.tile_pool(name="pp_y", bufs=4, space=bass.MemorySpace.PSUM) as pp_y,
                ):
                    for ht in range(4):
                        for qg in range(2):
                            psy = [pp_y.tile([128, 512], F32, tag="y", name=f"psy{i}")
                                   for i in range(4)]
                            for kvc in range(NCHK):
                                wo_t = dw.tile([128, 512], BF16, tag="wo")
                                nc.sync.dma_start(wo_t[:],
                                                  wo_d[kvc * 128:(kvc + 1) * 128,
                                                       ht * 512:(ht + 1) * 512])
                                for i in range(4):
                                    qc = qg * 4 + i
                                    nc.tensor.matmul(psy[i][:],
                                                     oT[:, kvc, qc * 128:(qc + 1) * 128],
                                                     wo_t[:],
                                                     start=(kvc == 0),
                                                     stop=(kvc == NCHK - 1))
                            for i in range(4):
                                qc = qg * 4 + i
                                xres = dx.tile([128, 512], F32, tag="xr")
                                nc.sync.dma_start(xres[:],
                                                  x_d[PC[qc] * 128:(PC[qc] + 1) * 128,
                                                      ht * 512:(ht + 1) * 512])
                                ysb = dy.tile([128, 512], F32, tag="y")
                                nc.vector.scalar_tensor_tensor(
                                    ysb[:], psy[i][:], recip[:, qc:qc + 1], xres[:],
                                    OP.mult, OP.add)
                                nc.sync.dma_start(y_d[qc * 128:(qc + 1) * 128,
                                                      ht * 512:(ht + 1) * 512], ysb[:])
    nc.compile()
    return nc


_NC_CACHE = None


def _get_nc():
    global _NC_CACHE
    if _NC_CACHE is None:
        _NC_CACHE = build()
    return _NC_CACHE


def make_in_maps(x, qkv, o_proj, gamma, beta):
    wqkv16 = np.ascontiguousarray(qkv).astype(ml_dtypes.bfloat16)
    wo16 = np.ascontiguousarray(o_proj).astype(ml_dtypes.bfloat16)
    gamma = np.ascontiguousarray(gamma, dtype=np.float32)
    beta = np.ascontiguousarray(beta, dtype=np.float32)
    in_maps, metas = [], []
    for c in range(8):
        b, h = c // 2, c % 2
        P = perm_chunks(h)
        ti = np.concatenate([np.arange(pc * 128, pc * 128 + 128) for pc in P])
        x_perm = np.ascontiguousarray(x[b][ti], dtype=np.float32)
        mask = np.zeros((4, 4, 128, 256), dtype=ml_dtypes.bfloat16)
        for e in range(4):
            qp = ti[512 * e:512 * e + 256]
            for cc in range(4):
                kp = ti[(4 * e + cc) * 128:(4 * e + cc + 1) * 128]
                mask[e, cc] = (kp[:, None] <= qp[None, :]).astype(ml_dtypes.bfloat16)
        in_maps.append({"x": x_perm, "wqkv": wqkv16, "wo": wo16,
                        "gamma": gamma, "beta": beta, "mask": mask})
        metas.append((b, ti))
    return in_maps, metas


def gather(results, metas, dtype):
    out = np.empty((B, S, H), dtype=dtype)
    qpos = np.concatenate([np.arange(512 * e, 512 * e + 256) for e in range(4)])
    for c, (b, ti) in enumerate(metas):
        out[b][ti[qpos]] = results[c]["y"]
    return out


def kernel(x, qkv, o_proj, gamma, beta, _trace=False):
    x = np.asarray(x, dtype=np.float32)
    nc = _get_nc()
    in_maps, metas = make_in_maps(x, qkv, o_proj, gamma, beta)
    res = run_bass_kernel_spmd(nc, in_maps, core_ids=list(range(8)), trace=_trace)
    out = gather(res.results, metas, np.float32)
    if _trace:
        kernel.last_result = res
    return out


# revision 8
# speedup vs baseline: 1.8239x; 1.8239x over previous
"""Trainium2 Bass kernel: pre-LN single-head causal attention + residual.

Reference computation (B=4, S=2048, H=K=2048, fp32):
    xn = LayerNorm(x) * gamma + beta
    q,k,v = xn @ qkv (split)
    out = causal_softmax(q k^T / sqrt(K)) @ v @ o_proj + x

Sharding: 8 cores = 4 batches x 2 query-halves. Each core gets its batch's
rows PERMUTED so that its query rows sit at fixed positions, arranged in 4
"classes" of 256 query rows whose causal key extent is 512*(e+1) rows --
a load-balanced folded-causal split with identical program shape on all
cores (pure SPMD; per-core behavior comes only from input data: the
permuted x and the causality masks).

On-device pipeline per core (all matmuls bf16, fp32 accumulate in PSUM):
  A0: LayerNorm stats (bn_stats) -> (x-mu)*rstd on ACT -> PE-transpose ->
      evacuate with gamma/beta fold -> x_norm^T bf16 [hid_p, tok_f]
  A1: v and k^T (spilled to DRAM), q^T resident (bf16)
  B:  scores^T = k^T-tiles^T @ q^T per class, exp on ACT (no max-subtract:
      |score*scale| < ~4 so fp32 exp is exact), causal mask multiply,
      denominators via ones-matmul
  C:  out^T = v-tiles^T @ attn^T (v streamed back from DRAM)
  D:  y = diag(1/sums) (out @ o_proj) + x  (normalization folded into the
      PSUM eviction as a per-partition scale; residual added in same op)
"""
import sys

import numpy as np

sys.path.insert(0, "/opt/trn_rl_repo")


def _install_ntff_hook():
    """Register the axon NTFF profile hook bass_utils expects (the image's
    antenv package lacks axon_hooks); degrades to no-op when unavailable."""
    import types
    if "antenv.axon_hooks" in sys.modules:
        return
    try:
        from trn_agent_boot.trn_boot import _ntff_profile_via_ctypes
        hook = _ntff_profile_via_ctypes("/opt/axon/libaxon_pjrt.so")
    except Exception:
        hook = None
    m = types.ModuleType("antenv.axon_hooks")
    m.get_axon_ntff_profile_hook = lambda: hook
    sys.modules["antenv.axon_hooks"] = m


_install_ntff_hook()

import ml_dtypes  # noqa: E402
import concourse.bass as bass  # noqa: E402
import concourse.tile as tile  # noqa: E402
from concourse import bacc, mybir  # noqa: E402
from concourse.bass_utils import run_bass_kernel_spmd  # noqa: E402

F32 = mybir.dt.float32
BF16 = mybir.dt.bfloat16
AF = mybir.ActivationFunctionType
OP = mybir.AluOpType

B, S, H, KEY = 4, 2048, 2048, 2048
NCHK = 16                 # 128-row chunks per sequence
EPS = 1e-5
SCALE = 1.0 / float(np.sqrt(KEY))
ABASE = [0, 4, 12, 24]    # attn^T tile base index per class
ATOT = 40                 # total k-chunk tiles across classes
PC = [0, 1, 4, 5, 8, 9, 12, 13]   # position chunks holding this core's q rows


def perm_chunks(h):
    out = []
    for e in range(4):
        out += [4 * e + 2 * h, 4 * e + 2 * h + 1,
                4 * e + 2 * (1 - h), 4 * e + 2 * (1 - h) + 1]
    return out


def build():
    nc = bacc.Bacc("TRN2", target_bir_lowering=False, debug=False)

    x_d = nc.dram_tensor("x", [S, H], F32, kind="ExternalInput")
    wqkv_d = nc.dram_tensor("wqkv", [H, 3 * KEY], BF16, kind="ExternalInput")
    wo_d = nc.dram_tensor("wo", [KEY, H], BF16, kind="ExternalInput")
    gamma_d = nc.dram_tensor("gamma", [H], F32, kind="ExternalInput")
    beta_d = nc.dram_tensor("beta", [H], F32, kind="ExternalInput")
    mask_d = nc.dram_tensor("mask", [4, 4, 128, 256], BF16, kind="ExternalInput")
    y_d = nc.dram_tensor("y", [1024, H], F32, kind="ExternalOutput")
    vsp_d = nc.dram_tensor("vsp", [NCHK, 128, KEY], BF16, kind="Internal")
    ssp_d = nc.dram_tensor("ssp", [4, 256], F32, kind="Internal")
    ksp_d = nc.dram_tensor("ksp", [NCHK, 128, S], BF16, kind="Internal")

    ident = nc.inline_tensor(np.eye(128, dtype=np.float32), name="ident")

    with tile.TileContext(nc) as tc:
        with (
            tc.tile_pool(name="small", bufs=1) as small,
            tc.tile_pool(name="p_main", bufs=1) as p_main,
        ):
            xnT = p_main.tile([128, NCHK, S], BF16)   # x_norm^T  [hid, tok]
            qT = p_main.tile([128, NCHK, 1024], BF16)  # q^T [key, class-packed q]
            recip = small.tile([128, 8], F32)         # 1/sums per q-chunk
            gcol = small.tile([128, NCHK], F32)       # gamma, [p, hc]
            bcol = small.tile([128, NCHK], F32)       # beta
            ones = small.tile([128, 1], BF16)
            id_sb = small.tile([128, 128], F32)

            nc.sync.dma_start(gcol[:], gamma_d[:].rearrange("(c p) -> p c", p=128))
            nc.sync.dma_start(bcol[:], beta_d[:].rearrange("(c p) -> p c", p=128))
            nc.sync.dma_start(id_sb[:], ident[:])
            nc.vector.memset(ones[:], 1.0)

            # ---------- A0: LN + transpose ----------
            with (
                nc.named_scope("ln_transpose"),
                tc.tile_pool(name="a0x", bufs=2) as a0x,
                tc.tile_pool(name="a0xp", bufs=5) as a0xp,
                tc.tile_pool(name="a0s", bufs=4) as a0s,
                tc.tile_pool(name="pp_tr", bufs=2, space=bass.MemorySpace.PSUM) as pp_tr,
            ):
                for tg in range(4):
                    xps = []
                    for i in range(4):
                        tci = tg * 4 + i
                        x_t = a0x.tile([128, H], F32, tag="x")
                        nc.sync.dma_start(x_t[:], x_d[tci * 128:(tci + 1) * 128, :])
                        st = a0s.tile([128, 4, 6], F32, tag="st")
                        for j in range(4):
                            nc.vector.bn_stats(st[:, j, :], x_t[:, j * 512:(j + 1) * 512])
                        ag = a0s.tile([128, 2], F32, tag="ag")
                        nc.vector.bn_aggr(ag[:], st[:])
                        veps = a0s.tile([128, 1], F32, tag="veps")
                        nc.vector.tensor_scalar_add(veps[:], ag[:, 1:2], EPS)
                        sq = a0s.tile([128, 1], F32, tag="sq")
                        nc.scalar.sqrt(sq[:], veps[:])
                        rstd = a0s.tile([128, 1], F32, tag="rstd")
                        nc.vector.reciprocal(rstd[:], sq[:])
                        nmr = a0s.tile([128, 1], F32, tag="nmr")
                        nc.vector.tensor_scalar(nmr[:], ag[:, 0:1], rstd[:], -1.0,
                                                OP.mult, OP.mult)
                        xp = a0xp.tile([128, H], F32, tag="xp")
                        nc.scalar.activation(xp[:], x_t[:], AF.Identity,
                                             bias=nmr[:], scale=rstd[:])
                        xps.append(xp)
                    for hc in range(NCHK):
                        ps = pp_tr.tile([128, 512], F32, tag="tr")
                        for i in range(4):
                            nc.tensor.transpose(ps[:, i * 128:(i + 1) * 128],
                                                xps[i][:, hc * 128:(hc + 1) * 128],
                                                id_sb[:])
                        dst = xnT[:, hc, tg * 512:(tg + 1) * 512]
                        if hc % 2 == 0:
                            nc.vector.tensor_scalar(dst, ps[:], gcol[:, hc:hc + 1],
                                                    bcol[:, hc:hc + 1], OP.mult, OP.add)
                        else:
                            nc.scalar.activation(dst, ps[:], AF.Identity,
                                                 bias=bcol[:, hc:hc + 1],
                                                 scale=gcol[:, hc:hc + 1])

            # ---------- A1a: v = xn @ Wv -> DRAM spill ----------
            with (
                nc.named_scope("v_proj"),
                tc.tile_pool(name="wv", bufs=1) as wvp,
                tc.tile_pool(name="vst", bufs=4) as vst,
                tc.tile_pool(name="pp_v", bufs=4, space=bass.MemorySpace.PSUM) as pp_v,
            ):
                wv_sb = wvp.tile([128, NCHK, KEY], BF16)
                for hc in range(NCHK):
                    nc.sync.dma_start(wv_sb[:, hc, :],
                                      wqkv_d[hc * 128:(hc + 1) * 128, 2 * KEY:3 * KEY])
                for tci in range(NCHK):
                    vs = vst.tile([128, KEY], BF16, tag="vs")
                    for kvt in range(4):
                        ps = pp_v.tile([128, 512], F32, tag="v")
                        for hc in range(NCHK):
                            nc.tensor.matmul(ps[:],
                                             xnT[:, hc, tci * 128:(tci + 1) * 128],
                                             wv_sb[:, hc, kvt * 512:(kvt + 1) * 512],
                                             start=(hc == 0), stop=(hc == NCHK - 1))
                        nc.vector.tensor_copy(vs[:, kvt * 512:(kvt + 1) * 512], ps[:])
                    nc.sync.dma_start(vsp_d[tci][:], vs[:])

            # ---------- A1b: q^T (resident), k^T (spilled) ----------
            with (
                nc.named_scope("qk_proj"),
                tc.tile_pool(name="wqk", bufs=8) as wqk,
                tc.tile_pool(name="kst", bufs=2) as kst,
                tc.tile_pool(name="pp_q", bufs=3, space=bass.MemorySpace.PSUM) as pp_q,
                tc.tile_pool(name="pp_k", bufs=5, space=bass.MemorySpace.PSUM) as pp_k,
            ):
                for kc in range(NCHK):
                    wq_s = wqk.tile([128, NCHK, 128], BF16, tag="wq")
                    nc.sync.dma_start(
                        wq_s[:], wqkv_d[:, kc * 128:(kc + 1) * 128]
                        .rearrange("(hc p) j -> p hc j", p=128))
                    wk_s = wqk.tile([128, NCHK, 128], BF16, tag="wk")
                    nc.sync.dma_start(
                        wk_s[:], wqkv_d[:, KEY + kc * 128:KEY + (kc + 1) * 128]
                        .rearrange("(hc p) j -> p hc j", p=128))
                    psq_a = pp_q.tile([128, 512], F32, tag="q")
                    psq_b = pp_q.tile([128, 512], F32, tag="q")
                    psk = [pp_k.tile([128, 512], F32, tag="k", name=f"psk{t4}")
                           for t4 in range(4)]
                    for hc in range(NCHK):
                        xr3 = xnT[:, hc, :].rearrange("p (g r) -> p g r", r=256)
                        st, sp = (hc == 0), (hc == NCHK - 1)
                        nc.tensor.matmul(psq_a[:], wq_s[:, hc, :], xr3[:, 0:3:2, :],
                                         start=st, stop=sp)
                        nc.tensor.matmul(psq_b[:], wq_s[:, hc, :], xr3[:, 4:7:2, :],
                                         start=st, stop=sp)
                        for t4 in range(4):
                            nc.tensor.matmul(psk[t4][:], wk_s[:, hc, :],
                                             xnT[:, hc, t4 * 512:(t4 + 1) * 512],
                                             start=st, stop=sp)
                    nc.vector.tensor_copy(qT[:, kc, 0:512], psq_a[:])
                    nc.vector.tensor_copy(qT[:, kc, 512:1024], psq_b[:])
                    ks = kst.tile([128, S], BF16, tag="ks")
                    for t4 in range(4):
                        if t4 % 2 == 0:
                            nc.scalar.copy(ks[:, t4 * 512:(t4 + 1) * 512], psk[t4][:])
                        else:
                            nc.vector.tensor_copy(ks[:, t4 * 512:(t4 + 1) * 512],
                                                  psk[t4][:])
                    nc.sync.dma_start(ksp_d[kc][:], ks[:])

            # ---------- B..D pools: attn^T and out^T ----------
            with tc.tile_pool(name="p_bc", bufs=1) as p_bc:
                aT = p_bc.tile([128, ATOT, 256], BF16)     # attn^T tiles
                oT = p_bc.tile([128, NCHK, 1024], BF16)    # out^T [kv, q]

                # ---------- B: scores^T -> exp -> mask -> attn^T, denominators ----
                with (
                    nc.named_scope("scores"),
                    tc.tile_pool(name="bk", bufs=3) as bk,
                    tc.tile_pool(name="bm", bufs=4) as bm,
                    tc.tile_pool(name="bs", bufs=4) as bs,
                    tc.tile_pool(name="pp_s", bufs=3, space=bass.MemorySpace.PSUM) as pp_s,
                    tc.tile_pool(name="pp_sum", bufs=4,
                                 space=bass.MemorySpace.PSUM) as pp_sum,
                ):
                    ps_sums = [pp_sum.tile([1, 256], F32, tag="sum", name=f"psum{e}")
                               for e in range(4)]
                    pending = []
                    for kch in range(NCHK):
                        kts = bk.tile([128, NCHK, 128], BF16, tag="kt")
                        nc.sync.dma_start(
                            kts[:], ksp_d[:, :, kch * 128:(kch + 1) * 128]
                            .rearrange("kc p j -> p kc j"))
                        this_round = []
                        for e in range(kch // 4, 4):
                            kext = 4 * (e + 1)
                            ps_s = pp_s.tile([128, 256], F32, tag="s")
                            for kc in range(NCHK):
                                nc.tensor.matmul(ps_s[:], kts[:, kc, :],
                                                 qT[:, kc, e * 256:(e + 1) * 256],
                                                 start=(kc == 0), stop=(kc == NCHK - 1))
                            dst = aT[:, ABASE[e] + kch, :]
                            if kch >= 4 * e:
                                tmp = bs.tile([128, 256], BF16, tag="exps")
                                nc.scalar.activation(tmp[:], ps_s[:], AF.Exp, scale=SCALE)
                                mt = bm.tile([128, 256], BF16, tag="mask")
                                nc.sync.dma_start(mt[:], mask_d[e, kch - 4 * e])
                                nc.vector.tensor_mul(dst, tmp[:], mt[:])
                            else:
                                nc.scalar.activation(dst, ps_s[:], AF.Exp, scale=SCALE)
                            this_round.append((e, kch, dst))
                        for e, kc0, src in pending:
                            nc.tensor.matmul(ps_sums[e][:], ones[:, 0:1], src,
                                             start=(kc0 == 0),
                                             stop=(kc0 == 4 * (e + 1) - 1))
                        pending = this_round
                    for e, kc0, src in pending:
                        nc.tensor.matmul(ps_sums[e][:], ones[:, 0:1], src,
                                         start=(kc0 == 0),
                                         stop=(kc0 == 4 * (e + 1) - 1))
                    for e in range(4):
                        srow = bs.tile([1, 256], F32, tag="srow")
                        nc.vector.tensor_copy(srow[:], ps_sums[e][:])
                        nc.sync.dma_start(ssp_d[e], srow[:])
                        scol = bs.tile([128, 2], F32, tag="scol")
                        nc.sync.dma_start(scol[:],
                                          ssp_d[e].rearrange("(j p) -> p j", p=128))
                        nc.vector.reciprocal(recip[:, 2 * e:2 * e + 2], scol[:])

                # ---------- C: out^T = v^T-chunks @ attn^T ----------
                with (
                    nc.named_scope("attn_v"),
                    tc.tile_pool(name="cv", bufs=3) as cv,
                    tc.tile_pool(name="pp_o", bufs=4, space=bass.MemorySpace.PSUM) as pp_o,
                ):
                    for kvc in range(NCHK):
                        vts = cv.tile([128, NCHK, 128], BF16, tag="vt")
                        nc.sync.dma_start(
                            vts[:], vsp_d[:, :, kvc * 128:(kvc + 1) * 128]
                            .rearrange("kch p j -> p kch j"))
                        for e in range(4):
                            kext = 4 * (e + 1)
                            ps_o = pp_o.tile([128, 256], F32, tag="o")
                            for kch in range(kext):
                                nc.tensor.matmul(ps_o[:], vts[:, kch, :],
                                                 aT[:, ABASE[e] + kch, :],
                                                 start=(kch == 0), stop=(kch == kext - 1))
                            if e % 2 == 0:
                                nc.scalar.copy(oT[:, kvc, e * 256:(e + 1) * 256],
                                               ps_o[:])
                            else:
                                nc.vector.tensor_copy(oT[:, kvc, e * 256:(e + 1) * 256],
                                                      ps_o[:])

                # ---------- D: y = diag(1/sums) (out @ Wo) + x ----------
                with (
                    nc.named_scope("o_proj"),
                    tc.tile_pool(name="dw", bufs=2) as dw,
                    tc.tile_pool(name="dx", bufs=4) as dx,
                    tc.tile_pool(name="dy", bufs=4) as dy,
                    tc.tile_pool(name="pp_y", bufs=4, space=bass.MemorySpace.PSUM) as pp_y,
                ):
                    for ht in range(4):
                        wo_s = dw.tile([128, NCHK, 512], BF16, tag="wo")
                        nc.sync.dma_start(wo_s[:],
                                          wo_d[:, ht * 512:(ht + 1) * 512]
                                          .rearrange("(kvc p) j -> p kvc j", p=128))
                        for qg in range(2):
                            psy = [pp_y.tile([128, 512], F32, tag="y", name=f"psy{i}")
                                   for i in range(4)]
                            for kvc in range(NCHK):
                                for i in range(4):
                                    qc = qg * 4 + i
                                    nc.tensor.matmul(psy[i][:],
                                                     oT[:, kvc, qc * 128:(qc + 1) * 128],
                                                     wo_s[:, kvc, :],
                                                     start=(kvc == 0),
                                                     stop=(kvc == NCHK - 1))
                            for i in range(4):
                                qc = qg * 4 + i
                                xres = dx.tile([128, 512], F32, tag="xr")
                                nc.sync.dma_start(xres[:],
                                                  x_d[PC[qc] * 128:(PC[qc] + 1) * 128,
                                                      ht * 512:(ht + 1) * 512])
                                ysb = dy.tile([128, 512], F32, tag="y")
                                nc.vector.scalar_tensor_tensor(
                                    ysb[:], psy[i][:], recip[:, qc:qc + 1], xres[:],
                                    OP.mult, OP.add)
                                nc.sync.dma_start(y_d[qc * 128:(qc + 1) * 128,
                                                      ht * 512:(ht + 1) * 512], ysb[:])
    nc.compile()
    return nc


_NC_CACHE = None


def _get_nc():
    global _NC_CACHE
    if _NC_CACHE is None:
        _NC_CACHE = build()
    return _NC_CACHE


def make_in_maps(x, qkv, o_proj, gamma, beta):
    wqkv16 = np.ascontiguousarray(qkv).astype(ml_dtypes.bfloat16)
    wo16 = np.ascontiguousarray(o_proj).astype(ml_dtypes.bfloat16)
    gamma = np.ascontiguousarray(gamma, dtype=np.float32)
    beta = np.ascontiguousarray(beta, dtype=np.float32)
    in_maps, metas = [], []
    for c in range(8):
        b, h = c // 2, c % 2
        P = perm_chunks(h)
        ti = np.concatenate([np.arange(pc * 128, pc * 128 + 128) for pc in P])
        x_perm = np.ascontiguousarray(x[b][ti], dtype=np.float32)
        mask = np.zeros((4, 4, 128, 256), dtype=ml_dtypes.bfloat16)
        for e in range(4):
            qp = ti[512 * e:512 * e + 256]
            for cc in range(4):
                kp = ti[(4 * e + cc) * 128:(4 * e + cc + 1) * 128]
                mask[e, cc] = (kp[:, None] <= qp[None, :]).astype(ml_dtypes.bfloat16)
        in_maps.append({"x": x_perm, "wqkv": wqkv16, "wo": wo16,
                        "gamma": gamma, "beta": beta, "mask": mask})
        metas.append((b, ti))
    return in_maps, metas


def gather(results, metas, dtype):
    out = np.empty((B, S, H), dtype=dtype)
    qpos = np.concatenate([np.arange(512 * e, 512 * e + 256) for e in range(4)])
    for c, (b, ti) in enumerate(metas):
        out[b][ti[qpos]] = results[c]["y"]
    return out


def kernel(x, qkv, o_proj, gamma, beta, _trace=False):
    x = np.asarray(x, dtype=np.float32)
    nc = _get_nc()
    in_maps, metas = make_in_maps(x, qkv, o_proj, gamma, beta)
    res = run_bass_kernel_spmd(nc, in_maps, core_ids=list(range(8)), trace=_trace)
    out = gather(res.results, metas, np.float32)
    if _trace:
        kernel.last_result = res
    return out


# revision 15
# speedup vs baseline: 2.0466x; 1.1221x over previous
"""Trainium2 Bass kernel: pre-LN single-head causal attention + residual.

Reference computation (B=4, S=2048, H=K=2048, fp32):
    xn = LayerNorm(x) * gamma + beta
    q,k,v = xn @ qkv (split)
    out = causal_softmax(q k^T / sqrt(K)) @ v @ o_proj + x

Sharding: 8 cores = 4 batches x 2 query-halves. Each core gets its batch's
rows PERMUTED so that its query rows sit at fixed positions, arranged in 4
"classes" of 256 query rows whose causal key extent is 512*(e+1) rows --
a load-balanced folded-causal split with identical program shape on all
cores (pure SPMD; per-core behavior comes only from input data: the
permuted x and the causality masks).

On-device pipeline per core (all matmuls bf16, fp32 accumulate in PSUM):
  A0: LayerNorm stats (bn_stats) -> (x-mu)*rstd on ACT -> PE-transpose ->
      evacuate with gamma/beta fold -> x_norm^T bf16 [hid_p, tok_f]
  A1: v and k^T (spilled to DRAM), q^T resident (bf16)
  B:  scores^T = k^T-tiles^T @ q^T per class, exp on ACT (no max-subtract:
      |score*scale| < ~4 so fp32 exp is exact), causal mask multiply,
      denominators via ones-matmul
  C:  out^T = v-tiles^T @ attn^T (v streamed back from DRAM)
  D:  y = diag(1/sums) (out @ o_proj) + x  (normalization folded into the
      PSUM eviction as a per-partition scale; residual added in same op)
"""
import os
import sys

import numpy as np

sys.path.insert(0, "/opt/trn_rl_repo")


def _install_ntff_hook():
    """Register the axon NTFF profile hook bass_utils expects (the image's
    antenv package lacks axon_hooks); degrades to no-op when unavailable."""
    import types
    if "antenv.axon_hooks" in sys.modules:
        return
    try:
        from trn_agent_boot.trn_boot import _ntff_profile_via_ctypes
        hook = _ntff_profile_via_ctypes("/opt/axon/libaxon_pjrt.so")
    except Exception:
        hook = None
    m = types.ModuleType("antenv.axon_hooks")
    m.get_axon_ntff_profile_hook = lambda: hook
    sys.modules["antenv.axon_hooks"] = m


_install_ntff_hook()

import ml_dtypes  # noqa: E402
import concourse.bass as bass  # noqa: E402
import concourse.tile as tile  # noqa: E402
from concourse import bacc, mybir  # noqa: E402
from concourse.bass_utils import run_bass_kernel_spmd  # noqa: E402

F32 = mybir.dt.float32
BF16 = mybir.dt.bfloat16
AF = mybir.ActivationFunctionType
OP = mybir.AluOpType

B, S, H, KEY = 4, 2048, 2048, 2048
NCHK = 16                 # 128-row chunks per sequence
EPS = 1e-5
SCALE = 1.0 / float(np.sqrt(KEY))
ABASE = [0, 4, 12, 24]    # attn^T tile base index per class
ATOT = 40                 # total k-chunk tiles across classes
PC = [0, 1, 4, 5, 8, 9, 12, 13]   # position chunks holding this core's q rows


def perm_chunks(h):
    out = []
    for e in range(4):
        out += [4 * e + 2 * h, 4 * e + 2 * h + 1,
                4 * e + 2 * (1 - h), 4 * e + 2 * (1 - h) + 1]
    return out


def build():
    nc = bacc.Bacc("TRN2", target_bir_lowering=False, debug=False, num_devices=8)

    x_d = nc.dram_tensor("x", [S, H], F32, kind="ExternalInput")
    wq_d = nc.dram_tensor("wq", [H, KEY], BF16, kind="ExternalInput")
    wk_d = nc.dram_tensor("wk", [H, KEY], BF16, kind="ExternalInput")
    wv_d = nc.dram_tensor("wv", [H, KEY], BF16, kind="ExternalInput")
    wo_d = nc.dram_tensor("wo", [KEY, H], BF16, kind="ExternalInput")
    gamma_d = nc.dram_tensor("gamma", [H], F32, kind="ExternalInput")
    beta_d = nc.dram_tensor("beta", [H], F32, kind="ExternalInput")
    mask_d = nc.dram_tensor("mask", [4, 4, 128, 256], BF16, kind="ExternalInput")
    y_d = nc.dram_tensor("y", [1024, H], F32, kind="ExternalOutput")
    DBG = bool(os.environ.get("K_DEBUG"))
    if DBG:
        dbg_k = nc.dram_tensor("dbg_k", [2, 2, 8, 128, 1024], BF16,
                               kind="ExternalOutput")
        dbg_v = nc.dram_tensor("dbg_v", [2, 8, 128, KEY], BF16,
                               kind="ExternalOutput")
        dbg_s = nc.dram_tensor("dbg_s", [4, 256], F32, kind="ExternalOutput")
        dbg_q = nc.dram_tensor("dbg_q", [NCHK, 128, 1024], BF16, kind="ExternalOutput")
    ssp_d = nc.dram_tensor("ssp", [4, 256], F32, kind="Internal")
    vsl_d = nc.dram_tensor("vsl", [8, 128, KEY], BF16, kind="Internal")
    vsp_d = nc.dram_tensor("vsp", [2, 8, 128, KEY], BF16, kind="Internal")
    ksla_d = nc.dram_tensor("ksla", [8, 128, 1024], BF16, kind="Internal")
    kslb_d = nc.dram_tensor("kslb", [8, 128, 1024], BF16, kind="Internal")
    kspa_d = nc.dram_tensor("kspa", [2, 8, 128, 1024], BF16, kind="Internal")
    kspb_d = nc.dram_tensor("kspb", [2, 8, 128, 1024], BF16, kind="Internal")
    GROUPS = [[2 * p, 2 * p + 1] for p in range(4)]

    ident = nc.inline_tensor(np.eye(128, dtype=np.float32), name="ident")

    with tile.TileContext(nc) as tc:
        with (
            tc.tile_pool(name="small", bufs=1) as small,
            tc.tile_pool(name="p_main", bufs=1) as p_main,
        ):
            xnT = p_main.tile([128, NCHK, S], BF16)   # x_norm^T  [hid, tok]
            recip = small.tile([128, 8], F32)         # 1/sums per q-chunk
            gcol = small.tile([128, NCHK], F32)       # gamma, [p, hc]
            bcol = small.tile([128, NCHK], F32)       # beta
            ones = small.tile([128, 1], BF16)
            id_sb = small.tile([128, 128], F32)

            nc.sync.dma_start(gcol[:], gamma_d[:].rearrange("(c p) -> p c", p=128))
            nc.sync.dma_start(bcol[:], beta_d[:].rearrange("(c p) -> p c", p=128))
            nc.sync.dma_start(id_sb[:], ident[:])
            nc.vector.memset(ones[:], 1.0)

            # ---------- A0: LN + transpose, interleaved with v-half ----------
            with (
                nc.named_scope("ln_transpose"),
                tc.tile_pool(name="a0x", bufs=2) as a0x,
                tc.tile_pool(name="a0xp", bufs=5) as a0xp,
                tc.tile_pool(name="a0s", bufs=4) as a0s,
                tc.tile_pool(name="wv", bufs=1) as wvp,
                tc.tile_pool(name="vst", bufs=2) as vst,
                tc.tile_pool(name="pp_tr", bufs=2, space=bass.MemorySpace.PSUM) as pp_tr,
                tc.tile_pool(name="pp_v", bufs=4, space=bass.MemorySpace.PSUM) as pp_v,
            ):
                wv_sb = wvp.tile([128, NCHK, KEY], BF16)
                for hc in range(NCHK):
                    nc.sync.dma_start(wv_sb[:, hc, :],
                                      wv_d[hc * 128:(hc + 1) * 128, :])
                for tg in range(4):
                    xps = []
                    for i in range(4):
                        tci = tg * 4 + i
                        x_t = a0x.tile([128, H], F32, tag="x")
                        nc.sync.dma_start(x_t[:], x_d[tci * 128:(tci + 1) * 128, :])
                        st = a0s.tile([128, 4, 6], F32, tag="st")
                        for j in range(4):
                            nc.vector.bn_stats(st[:, j, :], x_t[:, j * 512:(j + 1) * 512])
                        ag = a0s.tile([128, 2], F32, tag="ag")
                        nc.vector.bn_aggr(ag[:], st[:])
                        veps = a0s.tile([128, 1], F32, tag="veps")
                        nc.vector.tensor_scalar_add(veps[:], ag[:, 1:2], EPS)
                        sq = a0s.tile([128, 1], F32, tag="sq")
                        nc.scalar.sqrt(sq[:], veps[:])
                        rstd = a0s.tile([128, 1], F32, tag="rstd")
                        nc.vector.reciprocal(rstd[:], sq[:])
                        nmr = a0s.tile([128, 1], F32, tag="nmr")
                        nc.vector.tensor_scalar(nmr[:], ag[:, 0:1], rstd[:], -1.0,
                                                OP.mult, OP.mult)
                        xp = a0xp.tile([128, H], F32, tag="xp")
                        nc.scalar.activation(xp[:], x_t[:], AF.Identity,
                                             bias=nmr[:], scale=rstd[:])
                        xps.append(xp)
                    for hc in range(NCHK):
                        ps = pp_tr.tile([128, 512], F32, tag="tr")
                        for i in range(4):
                            nc.tensor.transpose(ps[:, i * 128:(i + 1) * 128],
                                                xps[i][:, hc * 128:(hc + 1) * 128],
                                                id_sb[:])
                        dst = xnT[:, hc, tg * 512:(tg + 1) * 512]
                        if hc % 2 == 0:
                            nc.vector.tensor_scalar(dst, ps[:], gcol[:, hc:hc + 1],
                                                    bcol[:, hc:hc + 1], OP.mult, OP.add)
                        else:
                            nc.scalar.activation(dst, ps[:], AF.Identity,
                                                 bias=bcol[:, hc:hc + 1],
                                                 scale=gcol[:, hc:hc + 1])
                    for i in range(2):
                        tci = tg * 4 + i          # own q-position chunks 4e, 4e+1
                        vs = vst.tile([128, KEY], BF16, tag="vs")
                        for kvt in range(4):
                            ps = pp_v.tile([128, 512], F32, tag="v")
                            for hc in range(NCHK):
                                nc.tensor.matmul(ps[:],
                                                 xnT[:, hc, tci * 128:(tci + 1) * 128],
                                                 wv_sb[:, hc, kvt * 512:(kvt + 1) * 512],
                                                 start=(hc == 0), stop=(hc == NCHK - 1))
                            nc.vector.tensor_copy(vs[:, kvt * 512:(kvt + 1) * 512],
                                                  ps[:])
                        nc.sync.dma_start(vsl_d[tg * 2 + i][:], vs[:])
            nc.gpsimd.collective_compute(
                "AllGather", OP.bypass, replica_groups=GROUPS,
                ins=[vsl_d.ap().opt()], outs=[vsp_d.ap().opt()])

            # ---------- pools for q^T, attn^T, out^T (A1b..D) ----------
            p_bc0 = tc.tile_pool(name="p_bc", bufs=1)
            p_bc = p_bc0.__enter__()
            qT = p_bc.tile([128, NCHK, 1024], BF16)   # q^T [key, class-packed q]
            aT = p_bc.tile([128, ATOT, 256], BF16)    # attn^T tiles
            oT = p_bc.tile([128, NCHK, 1024], BF16)   # out^T [kv, q]

            # ---------- A1b: q^T (resident), k^T (spilled) ----------
            with (
                nc.named_scope("qk_proj"),
                tc.tile_pool(name="wqk", bufs=3) as wqk,
                tc.tile_pool(name="kst", bufs=2) as kst,
                tc.tile_pool(name="pp_q", bufs=3, space=bass.MemorySpace.PSUM) as pp_q,
                tc.tile_pool(name="pp_k", bufs=5, space=bass.MemorySpace.PSUM) as pp_k,
            ):
                for kc in range(NCHK):
                    wk_s = wqk.tile([128, NCHK, 128], BF16, tag="wk")
                    nc.sync.dma_start(
                        wk_s[:], wk_d[:, kc * 128:(kc + 1) * 128]
                        .rearrange("(hc p) j -> p hc j", p=128))
                    psk_a = pp_k.tile([128, 512], F32, tag="k")
                    psk_b = pp_k.tile([128, 512], F32, tag="k")
                    for hc in range(NCHK):
                        xr3 = xnT[:, hc, :].rearrange("p (g r) -> p g r", r=256)
                        st, sp = (hc == 0), (hc == NCHK - 1)
                        nc.tensor.matmul(psk_a[:], wk_s[:, hc, :], xr3[:, 0:3:2, :],
                                         start=st, stop=sp)
                        nc.tensor.matmul(psk_b[:], wk_s[:, hc, :], xr3[:, 4:7:2, :],
                                         start=st, stop=sp)
                    ks = kst.tile([128, 1024], BF16, tag="ks")
                    nc.scalar.copy(ks[:, 0:512], psk_a[:])
                    nc.vector.tensor_copy(ks[:, 512:1024], psk_b[:])
                    ksl = ksla_d if kc < 8 else kslb_d
                    nc.sync.dma_start(ksl[kc % 8][:], ks[:])
                    if kc == 7:
                        nc.gpsimd.collective_compute(
                            "AllGather", OP.bypass, replica_groups=GROUPS,
                            ins=[ksla_d.ap().opt()], outs=[kspa_d.ap().opt()])
                nc.gpsimd.collective_compute(
                    "AllGather", OP.bypass, replica_groups=GROUPS,
                    ins=[kslb_d.ap().opt()], outs=[kspb_d.ap().opt()])
                for kc in range(NCHK):
                    wq_s = wqk.tile([128, NCHK, 128], BF16, tag="wq")
                    nc.sync.dma_start(
                        wq_s[:], wq_d[:, kc * 128:(kc + 1) * 128]
                        .rearrange("(hc p) j -> p hc j", p=128))
                    psq_a = pp_q.tile([128, 512], F32, tag="q")
                    psq_b = pp_q.tile([128, 512], F32, tag="q")
                    for hc in range(NCHK):
                        xr3 = xnT[:, hc, :].rearrange("p (g r) -> p g r", r=256)
                        st, sp = (hc == 0), (hc == NCHK - 1)
                        nc.tensor.matmul(psq_a[:], wq_s[:, hc, :], xr3[:, 0:3:2, :],
                                         start=st, stop=sp)
                        nc.tensor.matmul(psq_b[:], wq_s[:, hc, :], xr3[:, 4:7:2, :],
                                         start=st, stop=sp)
                    nc.vector.tensor_copy(qT[:, kc, 0:512], psq_a[:])
                    nc.vector.tensor_copy(qT[:, kc, 512:1024], psq_b[:])

            # ---------- B..D ----------
            if True:
                # ---------- B: scores^T -> exp -> mask -> attn^T, denominators ----
                with (
                    nc.named_scope("scores"),
                    tc.tile_pool(name="bk", bufs=3) as bk,
                    tc.tile_pool(name="bm", bufs=4) as bm,
                    tc.tile_pool(name="bs", bufs=4) as bs,
                    tc.tile_pool(name="pp_s", bufs=3, space=bass.MemorySpace.PSUM) as pp_s,
                    tc.tile_pool(name="pp_sum", bufs=4,
                                 space=bass.MemorySpace.PSUM) as pp_sum,
                ):
                    # tok-slot s = r*8 + lc: rank r's local tok chunk lc
                    # (true chunk 4*(lc//2) + 2r + lc%2); class e consumes
                    # slots with lc < 2(e+1).
                    ps_sums = [pp_sum.tile([1, 256], F32, tag="sum", name=f"psum{e}")
                               for e in range(4)]
                    pending = []
                    for s_ in range(NCHK):
                        r, lc = s_ // 8, s_ % 8
                        kts = bk.tile([128, NCHK, 128], BF16, tag="kt")
                        nc.sync.dma_start(
                            kts[:, 0:8, :],
                            kspa_d[r][:, :, lc * 128:(lc + 1) * 128]
                            .rearrange("kc p j -> p kc j"))
                        nc.sync.dma_start(
                            kts[:, 8:16, :],
                            kspb_d[r][:, :, lc * 128:(lc + 1) * 128]
                            .rearrange("kc p j -> p kc j"))
                        this_round = []
                        for e in range(lc // 2, 4):
                            ps_s = pp_s.tile([128, 256], F32, tag="s")
                            for kc in range(NCHK):
                                nc.tensor.matmul(ps_s[:], kts[:, kc, :],
                                                 qT[:, kc, e * 256:(e + 1) * 256],
                                                 start=(kc == 0), stop=(kc == NCHK - 1))
                            dst = aT[:, ABASE[e] + r * 2 * (e + 1) + lc, :]
                            if lc // 2 == e:
                                tmp = bs.tile([128, 256], BF16, tag="exps")
                                nc.scalar.activation(tmp[:], ps_s[:], AF.Exp, scale=SCALE)
                                mt = bm.tile([128, 256], BF16, tag="mask")
                                nc.sync.dma_start(mt[:], mask_d[e, r * 2 + lc % 2])
                                nc.vector.tensor_mul(dst, tmp[:], mt[:])
                            else:
                                nc.scalar.activation(dst, ps_s[:], AF.Exp, scale=SCALE)
                            this_round.append((e, s_, dst))
                        for e, sfrom, src in pending:
                            nc.tensor.matmul(ps_sums[e][:], ones[:, 0:1], src,
                                             start=(sfrom == 0),
                                             stop=(sfrom == 9 + 2 * e))
                        pending = this_round
                    for e, sfrom, src in pending:
                        nc.tensor.matmul(ps_sums[e][:], ones[:, 0:1], src,
                                         start=(sfrom == 0),
                                         stop=(sfrom == 9 + 2 * e))
                    for e in range(4):
                        srow = bs.tile([1, 256], F32, tag="srow")
                        nc.vector.tensor_copy(srow[:], ps_sums[e][:])
                        nc.sync.dma_start(ssp_d[e], srow[:])
                        scol = bs.tile([128, 2], F32, tag="scol")
                        nc.sync.dma_start(scol[:],
                                          ssp_d[e].rearrange("(j p) -> p j", p=128))
                        nc.vector.reciprocal(recip[:, 2 * e:2 * e + 2], scol[:])

                # ---------- C: out^T = v^T-chunks @ attn^T ----------
                with (
                    nc.named_scope("attn_v"),
                    tc.tile_pool(name="cv", bufs=3) as cv,
                    tc.tile_pool(name="pp_o", bufs=4, space=bass.MemorySpace.PSUM) as pp_o,
                ):
                    for kvc in range(NCHK):
                        vts = cv.tile([128, NCHK, 128], BF16, tag="vt")
                        for r in range(2):
                            nc.sync.dma_start(
                                vts[:, r * 8:(r + 1) * 8, :],
                                vsp_d[r][:, :, kvc * 128:(kvc + 1) * 128]
                                .rearrange("lc p j -> p lc j"))
                        for e in range(4):
                            slots = [(r, lc) for r in range(2)
                                     for lc in range(2 * (e + 1))]
                            ps_o = pp_o.tile([128, 256], F32, tag="o")
                            for si, (r, lc) in enumerate(slots):
                                nc.tensor.matmul(
                                    ps_o[:], vts[:, r * 8 + lc, :],
                                    aT[:, ABASE[e] + r * 2 * (e + 1) + lc, :],
                                    start=(si == 0), stop=(si == len(slots) - 1))
                            if e % 2 == 0:
                                nc.scalar.copy(oT[:, kvc, e * 256:(e + 1) * 256],
                                               ps_o[:])
                            else:
                                nc.vector.tensor_copy(oT[:, kvc, e * 256:(e + 1) * 256],
                                                      ps_o[:])

                # ---------- D: y = diag(1/sums) (out @ Wo) + x ----------
                with (
                    nc.named_scope("o_proj"),
                    tc.tile_pool(name="dw", bufs=2) as dw,
                    tc.tile_pool(name="dx", bufs=4) as dx,
                    tc.tile_pool(name="dy", bufs=4) as dy,
                    tc.tile_pool(name="pp_y", bufs=4, space=bass.MemorySpace.PSUM) as pp_y,
                ):
                    for ht in range(4):
                        wo_s = dw.tile([128, NCHK, 512], BF16, tag="wo")
                        nc.sync.dma_start(wo_s[:],
                                          wo_d[:, ht * 512:(ht + 1) * 512]
                                          .rearrange("(kvc p) j -> p kvc j", p=128))
                        for qg in range(2):
                            psy = [pp_y.tile([128, 512], F32, tag="y", name=f"psy{i}")
                                   for i in range(4)]
                            for kvc in range(NCHK):
                                for i in range(4):
                                    qc = qg * 4 + i
                                    nc.tensor.matmul(psy[i][:],
                                                     oT[:, kvc, qc * 128:(qc + 1) * 128],
                                                     wo_s[:, kvc, :],
                                                     start=(kvc == 0),
                                                     stop=(kvc == NCHK - 1))
                            for i in range(4):
                                qc = qg * 4 + i
                                xres = dx.tile([128, 512], F32, tag="xr")
                                nc.sync.dma_start(xres[:],
                                                  x_d[PC[qc] * 128:(PC[qc] + 1) * 128,
                                                      ht * 512:(ht + 1) * 512])
                                ysb = dy.tile([128, 512], F32, tag="y")
                                nc.vector.scalar_tensor_tensor(
                                    ysb[:], psy[i][:], recip[:, qc:qc + 1], xres[:],
                                    OP.mult, OP.add)
                                nc.sync.dma_start(y_d[qc * 128:(qc + 1) * 128,
                                                      ht * 512:(ht + 1) * 512], ysb[:])
                if DBG:
                    nc.sync.dma_start(dbg_k[0], kspa_d[:])
                    nc.sync.dma_start(dbg_k[1], kspb_d[:])
                    nc.sync.dma_start(dbg_v[:], vsp_d[:])
                    nc.sync.dma_start(dbg_s[:], ssp_d[:])
                    for kc in range(NCHK):
                        nc.sync.dma_start(dbg_q[kc], qT[:, kc, :])
            p_bc0.__exit__(None, None, None)
    nc.compile()
    return nc


_NC_CACHE = None


def _get_nc():
    global _NC_CACHE
    if _NC_CACHE is None:
        _NC_CACHE = build()
    return _NC_CACHE


def make_in_maps(x, qkv, o_proj, gamma, beta):
    qkv = np.asarray(qkv)
    wq16 = np.ascontiguousarray(qkv[:, :KEY]).astype(ml_dtypes.bfloat16)
    wk16 = np.ascontiguousarray(qkv[:, KEY:2 * KEY]).astype(ml_dtypes.bfloat16)
    wv16 = np.ascontiguousarray(qkv[:, 2 * KEY:]).astype(ml_dtypes.bfloat16)
    wo16 = np.ascontiguousarray(o_proj).astype(ml_dtypes.bfloat16)
    gamma = np.ascontiguousarray(gamma, dtype=np.float32)
    beta = np.ascontiguousarray(beta, dtype=np.float32)
    in_maps, metas = [], []
    for c in range(8):
        b, h = c // 2, c % 2
        P = perm_chunks(h)
        ti = np.concatenate([np.arange(pc * 128, pc * 128 + 128) for pc in P])
        x_perm = np.ascontiguousarray(x[b][ti], dtype=np.float32)
        # mask[e][2r+j]: k tok-slot (rank r, quad e, j) holds true chunk
        # 4e+2r+j; q col c of class e is true row ti[512e+c].
        mask = np.zeros((4, 4, 128, 256), dtype=ml_dtypes.bfloat16)
        for e in range(4):
            qp = ti[512 * e:512 * e + 256]
            for r in range(2):
                for j in range(2):
                    kp = (4 * e + 2 * r + j) * 128 + np.arange(128)
                    mask[e, 2 * r + j] = (kp[:, None] <= qp[None, :]).astype(
                        ml_dtypes.bfloat16)
        in_maps.append({"x": x_perm, "wq": wq16, "wk": wk16, "wv": wv16,
                        "wo": wo16, "gamma": gamma, "beta": beta, "mask": mask})
        metas.append((b, ti))
    return in_maps, metas


def gather(results, metas, dtype):
    out = np.empty((B, S, H), dtype=dtype)
    qpos = np.concatenate([np.arange(512 * e, 512 * e + 256) for e in range(4)])
    for c, (b, ti) in enumerate(metas):
        out[b][ti[qpos]] = results[c]["y"]
    return out


def kernel(x, qkv, o_proj, gamma, beta, _trace=False):
    x = np.asarray(x, dtype=np.float32)
    nc = _get_nc()
    in_maps, metas = make_in_maps(x, qkv, o_proj, gamma, beta)
    res = run_bass_kernel_spmd(nc, in_maps, core_ids=list(range(8)), trace=_trace)
    out = gather(res.results, metas, np.float32)
    if _trace:
        kernel.last_result = res
    return out


# revision 16
# speedup vs baseline: 2.2366x; 1.0928x over previous
"""Trainium2 Bass kernel: pre-LN single-head causal attention + residual.

Reference computation (B=4, S=2048, H=K=2048, fp32):
    xn = LayerNorm(x) * gamma + beta
    q,k,v = xn @ qkv (split)
    out = causal_softmax(q k^T / sqrt(K)) @ v @ o_proj + x

Sharding: 8 cores = 4 batches x 2 query-halves. Each core gets its batch's
rows PERMUTED so that its query rows sit at fixed positions, arranged in 4
"classes" of 256 query rows whose causal key extent is 512*(e+1) rows --
a load-balanced folded-causal split with identical program shape on all
cores (pure SPMD; per-core behavior comes only from input data: the
permuted x and the causality masks).

On-device pipeline per core (all matmuls bf16, fp32 accumulate in PSUM):
  A0: LayerNorm stats (bn_stats) -> (x-mu)*rstd on ACT -> PE-transpose ->
      evacuate with gamma/beta fold -> x_norm^T bf16 [hid_p, tok_f]
  A1: v and k^T (spilled to DRAM), q^T resident (bf16)
  B:  scores^T = k^T-tiles^T @ q^T per class, exp on ACT (no max-subtract:
      |score*scale| < ~4 so fp32 exp is exact), causal mask multiply,
      denominators via ones-matmul
  C:  out^T = v-tiles^T @ attn^T (v streamed back from DRAM)
  D:  y = diag(1/sums) (out @ o_proj) + x  (normalization folded into the
      PSUM eviction as a per-partition scale; residual added in same op)
"""
import os
import sys

import numpy as np

sys.path.insert(0, "/opt/trn_rl_repo")


def _install_ntff_hook():
    """Register the axon NTFF profile hook bass_utils expects (the image's
    antenv package lacks axon_hooks); degrades to no-op when unavailable."""
    import types
    if "antenv.axon_hooks" in sys.modules:
        return
    try:
        from trn_agent_boot.trn_boot import _ntff_profile_via_ctypes
        hook = _ntff_profile_via_ctypes("/opt/axon/libaxon_pjrt.so")
    except Exception:
        hook = None
    m = types.ModuleType("antenv.axon_hooks")
    m.get_axon_ntff_profile_hook = lambda: hook
    sys.modules["antenv.axon_hooks"] = m


_install_ntff_hook()

import ml_dtypes  # noqa: E402
import concourse.bass as bass  # noqa: E402
import concourse.tile as tile  # noqa: E402
from concourse import bacc, mybir  # noqa: E402
from concourse.bass_utils import run_bass_kernel_spmd  # noqa: E402

F32 = mybir.dt.float32
BF16 = mybir.dt.bfloat16
AF = mybir.ActivationFunctionType
OP = mybir.AluOpType

B, S, H, KEY = 4, 2048, 2048, 2048
NCHK = 16                 # 128-row chunks per sequence
EPS = 1e-5
SCALE = 1.0 / float(np.sqrt(KEY))
ABASE = [0, 4, 12, 24]    # attn^T tile base index per class
ATOT = 40                 # total k-chunk tiles across classes
PC = [0, 1, 4, 5, 8, 9, 12, 13]   # position chunks holding this core's q rows


def perm_chunks(h):
    out = []
    for e in range(4):
        out += [4 * e + 2 * h, 4 * e + 2 * h + 1,
                4 * e + 2 * (1 - h), 4 * e + 2 * (1 - h) + 1]
    return out


def build():
    nc = bacc.Bacc("TRN2", target_bir_lowering=False, debug=False, num_devices=8)

    x_d = nc.dram_tensor("x", [S, H], F32, kind="ExternalInput")
    wq_d = nc.dram_tensor("wq", [H, KEY], BF16, kind="ExternalInput")
    wk_d = nc.dram_tensor("wk", [H, KEY], BF16, kind="ExternalInput")
    wv_d = nc.dram_tensor("wv", [H, KEY], BF16, kind="ExternalInput")
    wo_d = nc.dram_tensor("wo", [KEY, H], BF16, kind="ExternalInput")
    gamma_d = nc.dram_tensor("gamma", [H], F32, kind="ExternalInput")
    beta_d = nc.dram_tensor("beta", [H], F32, kind="ExternalInput")
    mask_d = nc.dram_tensor("mask", [4, 4, 128, 256], BF16, kind="ExternalInput")
    y_d = nc.dram_tensor("y", [1024, H], F32, kind="ExternalOutput")
    DBG = bool(os.environ.get("K_DEBUG"))
    if DBG:
        dbg_k = nc.dram_tensor("dbg_k", [2, 2, 8, 128, 1024], BF16,
                               kind="ExternalOutput")
        dbg_v = nc.dram_tensor("dbg_v", [2, 8, 128, KEY], BF16,
                               kind="ExternalOutput")
        dbg_s = nc.dram_tensor("dbg_s", [4, 256], F32, kind="ExternalOutput")
        dbg_q = nc.dram_tensor("dbg_q", [NCHK, 128, 1024], BF16, kind="ExternalOutput")
    ssp_d = nc.dram_tensor("ssp", [4, 256], F32, kind="Internal")
    vsl_d = nc.dram_tensor("vsl", [8, 128, KEY], BF16, kind="Internal")
    vsp_d = nc.dram_tensor("vsp", [2, 8, 128, KEY], BF16, kind="Internal")
    ksla_d = nc.dram_tensor("ksla", [NCHK, 128, 512], BF16, kind="Internal")
    kslb_d = nc.dram_tensor("kslb", [NCHK, 128, 512], BF16, kind="Internal")
    kspa_d = nc.dram_tensor("kspa", [2, NCHK, 128, 512], BF16, kind="Internal")
    kspb_d = nc.dram_tensor("kspb", [2, NCHK, 128, 512], BF16, kind="Internal")
    GROUPS = [[2 * p, 2 * p + 1] for p in range(4)]

    import ml_dtypes as _mld
    ident = nc.inline_tensor(np.eye(128).astype(_mld.bfloat16), name="ident")

    with tile.TileContext(nc) as tc:
        with (
            tc.tile_pool(name="small", bufs=1) as small,
            tc.tile_pool(name="p_main", bufs=1) as p_main,
        ):
            xnT = p_main.tile([128, NCHK, S], BF16)   # x_norm^T  [hid, tok]
            recip = small.tile([128, 8], F32)         # 1/sums per q-chunk
            gcol = small.tile([128, NCHK], F32)       # gamma, [p, hc]
            bcol = small.tile([128, NCHK], F32)       # beta
            ones = small.tile([128, 1], BF16)
            id16_sb = small.tile([128, 128], BF16)

            nc.sync.dma_start(gcol[:], gamma_d[:].rearrange("(c p) -> p c", p=128))
            nc.sync.dma_start(bcol[:], beta_d[:].rearrange("(c p) -> p c", p=128))
            nc.sync.dma_start(id16_sb[:], ident[:])
            nc.vector.memset(ones[:], 1.0)

            # ---------- A0: LN + transpose, interleaved with v-half ----------
            with (
                nc.named_scope("ln_transpose"),
                tc.tile_pool(name="a0x", bufs=3) as a0x,
                tc.tile_pool(name="a0xp", bufs=8) as a0xp,
                tc.tile_pool(name="a0s", bufs=4) as a0s,
                tc.tile_pool(name="wv", bufs=1) as wvp,
                tc.tile_pool(name="vst", bufs=2) as vst,
                tc.tile_pool(name="pp_tr", bufs=2, space=bass.MemorySpace.PSUM) as pp_tr,
                tc.tile_pool(name="pp_v", bufs=4, space=bass.MemorySpace.PSUM) as pp_v,
            ):
                wv_sb = wvp.tile([128, NCHK, KEY], BF16)
                for hc in range(NCHK):
                    nc.sync.dma_start(wv_sb[:, hc, :],
                                      wv_d[hc * 128:(hc + 1) * 128, :])
                for tg in range(4):
                    xps = []
                    for i in range(4):
                        tci = tg * 4 + i
                        x_t = a0x.tile([128, H], F32, tag="x")
                        nc.sync.dma_start(x_t[:], x_d[tci * 128:(tci + 1) * 128, :])
                        st = a0s.tile([128, 4, 6], F32, tag="st")
                        for j in range(4):
                            nc.vector.bn_stats(st[:, j, :], x_t[:, j * 512:(j + 1) * 512])
                        ag = a0s.tile([128, 2], F32, tag="ag")
                        nc.vector.bn_aggr(ag[:], st[:])
                        veps = a0s.tile([128, 1], F32, tag="veps")
                        nc.vector.tensor_scalar_add(veps[:], ag[:, 1:2], EPS)
                        sq = a0s.tile([128, 1], F32, tag="sq")
                        nc.scalar.sqrt(sq[:], veps[:])
                        rstd = a0s.tile([128, 1], F32, tag="rstd")
                        nc.vector.reciprocal(rstd[:], sq[:])
                        nmr = a0s.tile([128, 1], F32, tag="nmr")
                        nc.vector.tensor_scalar(nmr[:], ag[:, 0:1], rstd[:], -1.0,
                                                OP.mult, OP.mult)
                        xp = a0xp.tile([128, H], BF16, tag="xp")
                        nc.vector.tensor_scalar(xp[:], x_t[:], rstd[:], nmr[:],
                                                OP.mult, OP.add)
                        xps.append(xp)
                    for hc in range(NCHK):
                        ps = pp_tr.tile([128, 512], BF16, tag="tr")
                        for i in range(4):
                            nc.tensor.transpose(ps[:, i * 128:(i + 1) * 128],
                                                xps[i][:, hc * 128:(hc + 1) * 128],
                                                id16_sb[:])
                        dst = xnT[:, hc, tg * 512:(tg + 1) * 512]
                        nc.scalar.activation(dst, ps[:], AF.Identity,
                                             bias=bcol[:, hc:hc + 1],
                                             scale=gcol[:, hc:hc + 1])
                    for i in range(2):
                        tci = tg * 4 + i          # own q-position chunks 4e, 4e+1
                        vs = vst.tile([128, KEY], BF16, tag="vs")
                        for kvt in range(4):
                            ps = pp_v.tile([128, 512], F32, tag="v")
                            for hc in range(NCHK):
                                nc.tensor.matmul(ps[:],
                                                 xnT[:, hc, tci * 128:(tci + 1) * 128],
                                                 wv_sb[:, hc, kvt * 512:(kvt + 1) * 512],
                                                 start=(hc == 0), stop=(hc == NCHK - 1))
                            nc.vector.tensor_copy(vs[:, kvt * 512:(kvt + 1) * 512],
                                                  ps[:])
                        nc.sync.dma_start(vsl_d[tg * 2 + i][:], vs[:])
            nc.gpsimd.collective_compute(
                "AllGather", OP.bypass, replica_groups=GROUPS,
                ins=[vsl_d.ap().opt()], outs=[vsp_d.ap().opt()])

            # ---------- pools for q^T, attn^T, out^T (A1b..D) ----------
            p_bc0 = tc.tile_pool(name="p_bc", bufs=1)
            p_bc = p_bc0.__enter__()
            qT = p_bc.tile([128, NCHK, 1024], BF16)   # q^T [key, class-packed q]
            aT = p_bc.tile([128, ATOT, 256], BF16)    # attn^T tiles
            oT = p_bc.tile([128, NCHK, 1024], BF16)   # out^T [kv, q]

            # ---------- A1b: q^T (resident), k^T (spilled) ----------
            with (
                nc.named_scope("qk_proj"),
                tc.tile_pool(name="wqk", bufs=3) as wqk,
                tc.tile_pool(name="kst", bufs=2) as kst,
                tc.tile_pool(name="pp_q", bufs=3, space=bass.MemorySpace.PSUM) as pp_q,
                tc.tile_pool(name="pp_k", bufs=5, space=bass.MemorySpace.PSUM) as pp_k,
            ):
                for half, (gsl, ksl, gout) in enumerate(
                        [(slice(0, 3), ksla_d, kspa_d), (slice(4, 7), kslb_d, kspb_d)]):
                    for kc in range(NCHK):
                        wk_s = wqk.tile([128, NCHK, 128], BF16, tag="wk")
                        nc.scalar.dma_start(
                            wk_s[:], wk_d[:, kc * 128:(kc + 1) * 128]
                            .rearrange("(hc p) j -> p hc j", p=128))
                        psk = pp_k.tile([128, 512], F32, tag="k")
                        for hc in range(NCHK):
                            xr3 = xnT[:, hc, :].rearrange("p (g r) -> p g r", r=256)
                            nc.tensor.matmul(psk[:], wk_s[:, hc, :],
                                             xr3[:, gsl.start:gsl.stop:2, :],
                                             start=(hc == 0), stop=(hc == NCHK - 1))
                        ks = kst.tile([128, 512], BF16, tag="ks")
                        if kc % 2 == 0:
                            nc.scalar.copy(ks[:], psk[:])
                        else:
                            nc.vector.tensor_copy(ks[:], psk[:])
                        nc.sync.dma_start(ksl[kc][:], ks[:])
                    nc.gpsimd.collective_compute(
                        "AllGather", OP.bypass, replica_groups=GROUPS,
                        ins=[ksl.ap().opt()], outs=[gout.ap().opt()])
                for kc in range(NCHK):
                    wq_s = wqk.tile([128, NCHK, 128], BF16, tag="wq")
                    nc.scalar.dma_start(
                        wq_s[:], wq_d[:, kc * 128:(kc + 1) * 128]
                        .rearrange("(hc p) j -> p hc j", p=128))
                    psq_a = pp_q.tile([128, 512], F32, tag="q")
                    psq_b = pp_q.tile([128, 512], F32, tag="q")
                    for hc in range(NCHK):
                        xr3 = xnT[:, hc, :].rearrange("p (g r) -> p g r", r=256)
                        st, sp = (hc == 0), (hc == NCHK - 1)
                        nc.tensor.matmul(psq_a[:], wq_s[:, hc, :], xr3[:, 0:3:2, :],
                                         start=st, stop=sp)
                        nc.tensor.matmul(psq_b[:], wq_s[:, hc, :], xr3[:, 4:7:2, :],
                                         start=st, stop=sp)
                    nc.vector.tensor_copy(qT[:, kc, 0:512], psq_a[:])
                    nc.vector.tensor_copy(qT[:, kc, 512:1024], psq_b[:])

            # ---------- B..D ----------
            if True:
                # ---------- B: scores^T -> exp -> mask -> attn^T, denominators ----
                with (
                    nc.named_scope("scores"),
                    tc.tile_pool(name="bk", bufs=3) as bk,
                    tc.tile_pool(name="bm", bufs=4) as bm,
                    tc.tile_pool(name="bs", bufs=4) as bs,
                    tc.tile_pool(name="pp_s", bufs=3, space=bass.MemorySpace.PSUM) as pp_s,
                    tc.tile_pool(name="pp_sum", bufs=4,
                                 space=bass.MemorySpace.PSUM) as pp_sum,
                ):
                    # tok-slot s = r*8 + lc: rank r's local tok chunk lc
                    # (true chunk 4*(lc//2) + 2r + lc%2); class e consumes
                    # slots with lc < 2(e+1).
                    ps_sums = [pp_sum.tile([1, 256], F32, tag="sum", name=f"psum{e}")
                               for e in range(4)]
                    pending = []
                    SLOT_ORDER = [(r, lc) for lc in range(8) for r in range(2)]
                    for r, lc in SLOT_ORDER:
                        # lc quads 0,1 live in kspa cols, quads 2,3 in kspb
                        src = kspa_d if lc < 4 else kspb_d
                        lcc = lc % 4
                        kts = bk.tile([128, NCHK, 128], BF16, tag="kt")
                        nc.sync.dma_start(
                            kts[:], src[r][:, :, lcc * 128:(lcc + 1) * 128]
                            .rearrange("kc p j -> p kc j"))
                        this_round = []
                        for e in range(lc // 2, 4):
                            ps_s = pp_s.tile([128, 256], F32, tag="s")
                            for kc in range(NCHK):
                                nc.tensor.matmul(ps_s[:], kts[:, kc, :],
                                                 qT[:, kc, e * 256:(e + 1) * 256],
                                                 start=(kc == 0), stop=(kc == NCHK - 1))
                            dst = aT[:, ABASE[e] + r * 2 * (e + 1) + lc, :]
                            if lc // 2 == e:
                                tmp = bs.tile([128, 256], BF16, tag="exps")
                                nc.scalar.activation(tmp[:], ps_s[:], AF.Exp, scale=SCALE)
                                mt = bm.tile([128, 256], BF16, tag="mask")
                                nc.sync.dma_start(mt[:], mask_d[e, r * 2 + lc % 2])
                                nc.vector.tensor_mul(dst, tmp[:], mt[:])
                            else:
                                nc.scalar.activation(dst, ps_s[:], AF.Exp, scale=SCALE)
                            # (r, lc) == (0, 0) is first for every class; class e
                            # ends at (1, 2e+1) in lc-major-then-r order.
                            this_round.append((e, (r, lc), dst))
                        for e, pos, src2 in pending:
                            nc.tensor.matmul(ps_sums[e][:], ones[:, 0:1], src2,
                                             start=(pos == (0, 0)),
                                             stop=(pos == (1, 2 * e + 1)))
                        pending = this_round
                    for e, pos, src2 in pending:
                        nc.tensor.matmul(ps_sums[e][:], ones[:, 0:1], src2,
                                         start=(pos == (0, 0)),
                                         stop=(pos == (1, 2 * e + 1)))
                    for e in range(4):
                        srow = bs.tile([1, 256], F32, tag="srow")
                        nc.vector.tensor_copy(srow[:], ps_sums[e][:])
                        nc.sync.dma_start(ssp_d[e], srow[:])
                        scol = bs.tile([128, 2], F32, tag="scol")
                        nc.sync.dma_start(scol[:],
                                          ssp_d[e].rearrange("(j p) -> p j", p=128))
                        nc.vector.reciprocal(recip[:, 2 * e:2 * e + 2], scol[:])

                # ---------- C: out^T = v^T-chunks @ attn^T ----------
                with (
                    nc.named_scope("attn_v"),
                    tc.tile_pool(name="cv", bufs=3) as cv,
                    tc.tile_pool(name="pp_o", bufs=2, space=bass.MemorySpace.PSUM) as pp_o,
                ):
                    for kvc in range(NCHK):
                        vts = cv.tile([128, NCHK, 128], BF16, tag="vt")
                        for r in range(2):
                            nc.sync.dma_start(
                                vts[:, r * 8:(r + 1) * 8, :],
                                vsp_d[r][:, :, kvc * 128:(kvc + 1) * 128]
                                .rearrange("lc p j -> p lc j"))
                        for e in range(4):
                            slots = [(r, lc) for r in range(2)
                                     for lc in range(2 * (e + 1))]
                            ps_o = pp_o.tile([128, 256], F32, tag="o")
                            for si, (r, lc) in enumerate(slots):
                                nc.tensor.matmul(
                                    ps_o[:], vts[:, r * 8 + lc, :],
                                    aT[:, ABASE[e] + r * 2 * (e + 1) + lc, :],
                                    start=(si == 0), stop=(si == len(slots) - 1))
                            if e % 2 == 0:
                                nc.scalar.copy(oT[:, kvc, e * 256:(e + 1) * 256],
                                               ps_o[:])
                            else:
                                nc.vector.tensor_copy(oT[:, kvc, e * 256:(e + 1) * 256],
                                                      ps_o[:])

                # ---------- D: y = diag(1/sums) (out @ Wo) + x ----------
                with (
                    nc.named_scope("o_proj"),
                    tc.tile_pool(name="dw", bufs=2) as dw,
                    tc.tile_pool(name="dx", bufs=4) as dx,
                    tc.tile_pool(name="dy", bufs=4) as dy,
                    tc.tile_pool(name="pp_y", bufs=6, space=bass.MemorySpace.PSUM) as pp_y,
                ):
                    for ht in range(4):
                        wo_s = dw.tile([128, NCHK, 512], BF16, tag="wo")
                        nc.sync.dma_start(wo_s[:],
                                          wo_d[:, ht * 512:(ht + 1) * 512]
                                          .rearrange("(kvc p) j -> p kvc j", p=128))
                        for qg in range(2):
                            psy = [pp_y.tile([128, 512], F32, tag="y", name=f"psy{i}")
                                   for i in range(4)]
                            for kvc in range(NCHK):
                                for i in range(4):
                                    qc = qg * 4 + i
                                    nc.tensor.matmul(psy[i][:],
                                                     oT[:, kvc, qc * 128:(qc + 1) * 128],
                                                     wo_s[:, kvc, :],
                                                     start=(kvc == 0),
                                                     stop=(kvc == NCHK - 1))
                            for i in range(4):
                                qc = qg * 4 + i
                                xres = dx.tile([128, 512], F32, tag="xr")
                                nc.sync.dma_start(xres[:],
                                                  x_d[PC[qc] * 128:(PC[qc] + 1) * 128,
                                                      ht * 512:(ht + 1) * 512])
                                ysb = dy.tile([128, 512], F32, tag="y")
                                nc.vector.scalar_tensor_tensor(
                                    ysb[:], psy[i][:], recip[:, qc:qc + 1], xres[:],
                                    OP.mult, OP.add)
                                nc.sync.dma_start(y_d[qc * 128:(qc + 1) * 128,
                                                      ht * 512:(ht + 1) * 512], ysb[:])
                if DBG:
                    nc.sync.dma_start(dbg_k[0], kspa_d[:])
                    nc.sync.dma_start(dbg_k[1], kspb_d[:])
                    nc.sync.dma_start(dbg_v[:], vsp_d[:])
                    nc.sync.dma_start(dbg_s[:], ssp_d[:])
                    for kc in range(NCHK):
                        nc.sync.dma_start(dbg_q[kc], qT[:, kc, :])
            p_bc0.__exit__(None, None, None)
    nc.compile()
    return nc


_NC_CACHE = None


def _get_nc():
    global _NC_CACHE
    if _NC_CACHE is None:
        _NC_CACHE = build()
    return _NC_CACHE


def make_in_maps(x, qkv, o_proj, gamma, beta):
    qkv = np.asarray(qkv)
    wq16 = np.ascontiguousarray(qkv[:, :KEY]).astype(ml_dtypes.bfloat16)
    wk16 = np.ascontiguousarray(qkv[:, KEY:2 * KEY]).astype(ml_dtypes.bfloat16)
    wv16 = np.ascontiguousarray(qkv[:, 2 * KEY:]).astype(ml_dtypes.bfloat16)
    wo16 = np.ascontiguousarray(o_proj).astype(ml_dtypes.bfloat16)
    gamma = np.ascontiguousarray(gamma, dtype=np.float32)
    beta = np.ascontiguousarray(beta, dtype=np.float32)
    in_maps, metas = [], []
    for c in range(8):
        b, h = c // 2, c % 2
        P = perm_chunks(h)
        ti = np.concatenate([np.arange(pc * 128, pc * 128 + 128) for pc in P])
        x_perm = np.ascontiguousarray(x[b][ti], dtype=np.float32)
        # mask[e][2r+j]: k tok-slot (rank r, quad e, j) holds true chunk
        # 4e+2r+j; q col c of class e is true row ti[512e+c].
        mask = np.zeros((4, 4, 128, 256), dtype=ml_dtypes.bfloat16)
        for e in range(4):
            qp = ti[512 * e:512 * e + 256]
            for r in range(2):
                for j in range(2):
                    kp = (4 * e + 2 * r + j) * 128 + np.arange(128)
                    mask[e, 2 * r + j] = (kp[:, None] <= qp[None, :]).astype(
                        ml_dtypes.bfloat16)
        in_maps.append({"x": x_perm, "wq": wq16, "wk": wk16, "wv": wv16,
                        "wo": wo16, "gamma": gamma, "beta": beta, "mask": mask})
        metas.append((b, ti))
    return in_maps, metas


def gather(results, metas, dtype):
    out = np.empty((B, S, H), dtype=dtype)
    qpos = np.concatenate([np.arange(512 * e, 512 * e + 256) for e in range(4)])
    for c, (b, ti) in enumerate(metas):
        out[b][ti[qpos]] = results[c]["y"]
    return out


def kernel(x, qkv, o_proj, gamma, beta, _trace=False):
    x = np.asarray(x, dtype=np.float32)
    nc = _get_nc()
    in_maps, metas = make_in_maps(x, qkv, o_proj, gamma, beta)
    res = run_bass_kernel_spmd(nc, in_maps, core_ids=list(range(8)), trace=_trace)
    out = gather(res.results, metas, np.float32)
    if _trace:
        kernel.last_result = res
    return out


# revision 17
# speedup vs baseline: 2.2930x; 1.0252x over previous
"""Trainium2 Bass kernel: pre-LN single-head causal attention + residual.

Reference computation (B=4, S=2048, H=K=2048, fp32):
    xn = LayerNorm(x) * gamma + beta
    q,k,v = xn @ qkv (split)
    out = causal_softmax(q k^T / sqrt(K)) @ v @ o_proj + x

Sharding: 8 cores = 4 batches x 2 query-halves. Each core gets its batch's
rows PERMUTED so that its query rows sit at fixed positions, arranged in 4
"classes" of 256 query rows whose causal key extent is 512*(e+1) rows --
a load-balanced folded-causal split with identical program shape on all
cores (pure SPMD; per-core behavior comes only from input data: the
permuted x and the causality masks).

On-device pipeline per core (all matmuls bf16, fp32 accumulate in PSUM):
  A0: LayerNorm stats (bn_stats) -> (x-mu)*rstd on ACT -> PE-transpose ->
      evacuate with gamma/beta fold -> x_norm^T bf16 [hid_p, tok_f]
  A1: v and k^T (spilled to DRAM), q^T resident (bf16)
  B:  scores^T = k^T-tiles^T @ q^T per class, exp on ACT (no max-subtract:
      |score*scale| < ~4 so fp32 exp is exact), causal mask multiply,
      denominators via ones-matmul
  C:  out^T = v-tiles^T @ attn^T (v streamed back from DRAM)
  D:  y = diag(1/sums) (out @ o_proj) + x  (normalization folded into the
      PSUM eviction as a per-partition scale; residual added in same op)
"""
import os
import sys

import numpy as np

sys.path.insert(0, "/opt/trn_rl_repo")


def _install_ntff_hook():
    """Register the axon NTFF profile hook bass_utils expects (the image's
    antenv package lacks axon_hooks); degrades to no-op when unavailable."""
    import types
    if "antenv.axon_hooks" in sys.modules:
        return
    try:
        from trn_agent_boot.trn_boot import _ntff_profile_via_ctypes
        hook = _ntff_profile_via_ctypes("/opt/axon/libaxon_pjrt.so")
    except Exception:
        hook = None
    m = types.ModuleType("antenv.axon_hooks")
    m.get_axon_ntff_profile_hook = lambda: hook
    sys.modules["antenv.axon_hooks"] = m


_install_ntff_hook()

import ml_dtypes  # noqa: E402
import concourse.bass as bass  # noqa: E402
import concourse.tile as tile  # noqa: E402
from concourse import bacc, mybir  # noqa: E402
from concourse.bass_utils import run_bass_kernel_spmd  # noqa: E402

F32 = mybir.dt.float32
BF16 = mybir.dt.bfloat16
AF = mybir.ActivationFunctionType
OP = mybir.AluOpType

B, S, H, KEY = 4, 2048, 2048, 2048
NCHK = 16                 # 128-row chunks per sequence
EPS = 1e-5
SCALE = 1.0 / float(np.sqrt(KEY))
ABASE = [0, 4, 12, 24]    # attn^T tile base index per class
ATOT = 40                 # total k-chunk tiles across classes
PC = [0, 1, 4, 5, 8, 9, 12, 13]   # position chunks holding this core's q rows


def perm_chunks(h):
    out = []
    for e in range(4):
        out += [4 * e + 2 * h, 4 * e + 2 * h + 1,
                4 * e + 2 * (1 - h), 4 * e + 2 * (1 - h) + 1]
    return out


def build():
    nc = bacc.Bacc("TRN2", target_bir_lowering=False, debug=False, num_devices=8)

    x_d = nc.dram_tensor("x", [S, H], F32, kind="ExternalInput")
    wq_d = nc.dram_tensor("wq", [H, KEY], BF16, kind="ExternalInput")
    wk_d = nc.dram_tensor("wk", [H, KEY], BF16, kind="ExternalInput")
    wv_d = nc.dram_tensor("wv", [H, KEY], BF16, kind="ExternalInput")
    wo_d = nc.dram_tensor("wo", [KEY, H], BF16, kind="ExternalInput")
    gamma_d = nc.dram_tensor("gamma", [H], F32, kind="ExternalInput")
    beta_d = nc.dram_tensor("beta", [H], F32, kind="ExternalInput")
    mask_d = nc.dram_tensor("mask", [4, 4, 128, 256], BF16, kind="ExternalInput")
    y_d = nc.dram_tensor("y", [1024, H], F32, kind="ExternalOutput")
    DBG = bool(os.environ.get("K_DEBUG"))
    if DBG:
        dbg_k = nc.dram_tensor("dbg_k", [2, 2, 8, 128, 1024], BF16,
                               kind="ExternalOutput")
        dbg_v = nc.dram_tensor("dbg_v", [2, 8, 128, KEY], BF16,
                               kind="ExternalOutput")
        dbg_s = nc.dram_tensor("dbg_s", [4, 256], F32, kind="ExternalOutput")
        dbg_q = nc.dram_tensor("dbg_q", [NCHK, 128, 1024], BF16, kind="ExternalOutput")
    ssp_d = nc.dram_tensor("ssp", [4, 256], F32, kind="Internal")
    vsl_d = nc.dram_tensor("vsl", [8, 128, KEY], BF16, kind="Internal")
    vsp_d = nc.dram_tensor("vsp", [2, 8, 128, KEY], BF16, kind="Internal")
    ksla_d = nc.dram_tensor("ksla", [8, 128, 1024], BF16, kind="Internal")
    kslb_d = nc.dram_tensor("kslb", [8, 128, 1024], BF16, kind="Internal")
    kspa_d = nc.dram_tensor("kspa", [2, 8, 128, 1024], BF16, kind="Internal")
    kspb_d = nc.dram_tensor("kspb", [2, 8, 128, 1024], BF16, kind="Internal")
    GROUPS = [[2 * p, 2 * p + 1] for p in range(4)]

    import ml_dtypes as _mld
    ident = nc.inline_tensor(np.eye(128).astype(_mld.bfloat16), name="ident")

    with tile.TileContext(nc) as tc:
        with (
            tc.tile_pool(name="small", bufs=1) as small,
            tc.tile_pool(name="p_main", bufs=1) as p_main,
        ):
            xnT = p_main.tile([128, NCHK, S], BF16)   # x_norm^T  [hid, tok]
            recip = small.tile([128, 8], F32)         # 1/sums per q-chunk
            gcol = small.tile([128, NCHK], F32)       # gamma, [p, hc]
            bcol = small.tile([128, NCHK], F32)       # beta
            ones = small.tile([128, 1], BF16)
            id16_sb = small.tile([128, 128], BF16)

            nc.sync.dma_start(gcol[:], gamma_d[:].rearrange("(c p) -> p c", p=128))
            nc.sync.dma_start(bcol[:], beta_d[:].rearrange("(c p) -> p c", p=128))
            nc.sync.dma_start(id16_sb[:], ident[:])
            nc.vector.memset(ones[:], 1.0)

            # ---------- A0: LN + transpose, interleaved with v-half ----------
            with (
                nc.named_scope("ln_transpose"),
                tc.tile_pool(name="a0x", bufs=3) as a0x,
                tc.tile_pool(name="a0xp", bufs=8) as a0xp,
                tc.tile_pool(name="a0s", bufs=4) as a0s,
                tc.tile_pool(name="wv", bufs=1) as wvp,
                tc.tile_pool(name="vst", bufs=2) as vst,
                tc.tile_pool(name="pp_tr", bufs=2, space=bass.MemorySpace.PSUM) as pp_tr,
                tc.tile_pool(name="pp_v", bufs=4, space=bass.MemorySpace.PSUM) as pp_v,
            ):
                wv_sb = wvp.tile([128, NCHK, KEY], BF16)
                for hc in range(NCHK):
                    nc.sync.dma_start(wv_sb[:, hc, :],
                                      wv_d[hc * 128:(hc + 1) * 128, :])
                for tg in range(4):
                    xps = []
                    for i in range(4):
                        tci = tg * 4 + i
                        x_t = a0x.tile([128, H], F32, tag="x")
                        nc.sync.dma_start(x_t[:], x_d[tci * 128:(tci + 1) * 128, :])
                        st = a0s.tile([128, 4, 6], F32, tag="st")
                        for j in range(4):
                            nc.vector.bn_stats(st[:, j, :], x_t[:, j * 512:(j + 1) * 512])
                        ag = a0s.tile([128, 2], F32, tag="ag")
                        nc.vector.bn_aggr(ag[:], st[:])
                        veps = a0s.tile([128, 1], F32, tag="veps")
                        nc.vector.tensor_scalar_add(veps[:], ag[:, 1:2], EPS)
                        sq = a0s.tile([128, 1], F32, tag="sq")
                        nc.scalar.sqrt(sq[:], veps[:])
                        rstd = a0s.tile([128, 1], F32, tag="rstd")
                        nc.vector.reciprocal(rstd[:], sq[:])
                        nmr = a0s.tile([128, 1], F32, tag="nmr")
                        nc.vector.tensor_scalar(nmr[:], ag[:, 0:1], rstd[:], -1.0,
                                                OP.mult, OP.mult)
                        xp = a0xp.tile([128, H], BF16, tag="xp")
                        nc.vector.tensor_scalar(xp[:], x_t[:], rstd[:], nmr[:],
                                                OP.mult, OP.add)
                        xps.append(xp)
                    for hc in range(NCHK):
                        ps = pp_tr.tile([128, 512], BF16, tag="tr")
                        for i in range(4):
                            nc.tensor.transpose(ps[:, i * 128:(i + 1) * 128],
                                                xps[i][:, hc * 128:(hc + 1) * 128],
                                                id16_sb[:])
                        dst = xnT[:, hc, tg * 512:(tg + 1) * 512]
                        nc.scalar.activation(dst, ps[:], AF.Identity,
                                             bias=bcol[:, hc:hc + 1],
                                             scale=gcol[:, hc:hc + 1])
                    for i in range(2):
                        tci = tg * 4 + i          # own q-position chunks 4e, 4e+1
                        vs = vst.tile([128, KEY], BF16, tag="vs")
                        for kvt in range(4):
                            ps = pp_v.tile([128, 512], F32, tag="v")
                            for hc in range(NCHK):
                                nc.tensor.matmul(ps[:],
                                                 xnT[:, hc, tci * 128:(tci + 1) * 128],
                                                 wv_sb[:, hc, kvt * 512:(kvt + 1) * 512],
                                                 start=(hc == 0), stop=(hc == NCHK - 1))
                            nc.vector.tensor_copy(vs[:, kvt * 512:(kvt + 1) * 512],
                                                  ps[:])
                        nc.sync.dma_start(vsl_d[tg * 2 + i][:], vs[:])
            nc.gpsimd.collective_compute(
                "AllGather", OP.bypass, replica_groups=GROUPS,
                ins=[vsl_d.ap().opt()], outs=[vsp_d.ap().opt()])

            # ---------- pools for q^T, attn^T, out^T (A1b..D) ----------
            p_bc0 = tc.tile_pool(name="p_bc", bufs=1)
            p_bc = p_bc0.__enter__()
            qT = p_bc.tile([128, NCHK, 1024], BF16)   # q^T [key, class-packed q]
            aT = p_bc.tile([128, ATOT, 256], BF16)    # attn^T tiles
            oT = p_bc.tile([128, NCHK, 1024], BF16)   # out^T [kv, q]

            # ---------- A1b: q^T (resident), k^T (spilled) ----------
            with (
                nc.named_scope("qk_proj"),
                tc.tile_pool(name="wqk", bufs=6) as wqk,
                tc.tile_pool(name="kst", bufs=2) as kst,
                tc.tile_pool(name="pp_q", bufs=3, space=bass.MemorySpace.PSUM) as pp_q,
                tc.tile_pool(name="pp_k", bufs=5, space=bass.MemorySpace.PSUM) as pp_k,
            ):
                wk_tiles = []
                for kc in range(NCHK):
                    wk_s = wqk.tile([128, NCHK, 128], BF16, tag="wk",
                                    name=f"wk_s{kc}")
                    nc.sync.dma_start(
                        wk_s[:], wk_d[:, kc * 128:(kc + 1) * 128]
                        .rearrange("(hc p) j -> p hc j", p=128))
                    wk_tiles.append(wk_s)
                for kc in range(NCHK):
                    wk_s = wk_tiles[kc]
                    psk_a = pp_k.tile([128, 512], F32, tag="k")
                    psk_b = pp_k.tile([128, 512], F32, tag="k")
                    for hc in range(NCHK):
                        xr3 = xnT[:, hc, :].rearrange("p (g r) -> p g r", r=256)
                        st, sp = (hc == 0), (hc == NCHK - 1)
                        nc.tensor.matmul(psk_a[:], wk_s[:, hc, :], xr3[:, 0:3:2, :],
                                         start=st, stop=sp)
                        nc.tensor.matmul(psk_b[:], wk_s[:, hc, :], xr3[:, 4:7:2, :],
                                         start=st, stop=sp)
                    ks = kst.tile([128, 1024], BF16, tag="ks")
                    nc.scalar.copy(ks[:, 0:512], psk_a[:])
                    nc.vector.tensor_copy(ks[:, 512:1024], psk_b[:])
                    ksl = ksla_d if kc < 8 else kslb_d
                    nc.sync.dma_start(ksl[kc % 8][:], ks[:])
                    if kc == 7:
                        nc.gpsimd.collective_compute(
                            "AllGather", OP.bypass, replica_groups=GROUPS,
                            ins=[ksla_d.ap().opt()], outs=[kspa_d.ap().opt()])
                nc.gpsimd.collective_compute(
                    "AllGather", OP.bypass, replica_groups=GROUPS,
                    ins=[kslb_d.ap().opt()], outs=[kspb_d.ap().opt()])
                wq_tiles = []
                for kc in range(NCHK):
                    wq_s = wqk.tile([128, NCHK, 128], BF16, tag="wq",
                                    name=f"wq_s{kc}")
                    nc.sync.dma_start(
                        wq_s[:], wq_d[:, kc * 128:(kc + 1) * 128]
                        .rearrange("(hc p) j -> p hc j", p=128))
                    wq_tiles.append(wq_s)
                for kc in range(NCHK):
                    wq_s = wq_tiles[kc]
                    psq_a = pp_q.tile([128, 512], F32, tag="q")
                    psq_b = pp_q.tile([128, 512], F32, tag="q")
                    for hc in range(NCHK):
                        xr3 = xnT[:, hc, :].rearrange("p (g r) -> p g r", r=256)
                        st, sp = (hc == 0), (hc == NCHK - 1)
                        nc.tensor.matmul(psq_a[:], wq_s[:, hc, :], xr3[:, 0:3:2, :],
                                         start=st, stop=sp)
                        nc.tensor.matmul(psq_b[:], wq_s[:, hc, :], xr3[:, 4:7:2, :],
                                         start=st, stop=sp)
                    nc.vector.tensor_copy(qT[:, kc, 0:512], psq_a[:])
                    nc.vector.tensor_copy(qT[:, kc, 512:1024], psq_b[:])

            # ---------- B..D ----------
            if True:
                # ---------- B: scores^T -> exp -> mask -> attn^T, denominators ----
                with (
                    nc.named_scope("scores"),
                    tc.tile_pool(name="bk", bufs=3) as bk,
                    tc.tile_pool(name="bm", bufs=4) as bm,
                    tc.tile_pool(name="bs", bufs=4) as bs,
                    tc.tile_pool(name="pp_s", bufs=3, space=bass.MemorySpace.PSUM) as pp_s,
                    tc.tile_pool(name="pp_sum", bufs=4,
                                 space=bass.MemorySpace.PSUM) as pp_sum,
                ):
                    # tok-slot s = r*8 + lc: rank r's local tok chunk lc
                    # (true chunk 4*(lc//2) + 2r + lc%2); class e consumes
                    # slots with lc < 2(e+1).
                    ps_sums = [pp_sum.tile([1, 256], F32, tag="sum", name=f"psum{e}")
                               for e in range(4)]
                    pending = []
                    SLOT_ORDER = [(r, lc) for lc in range(8) for r in range(2)]
                    for r, lc in SLOT_ORDER:
                        kts = bk.tile([128, NCHK, 128], BF16, tag="kt")
                        nc.sync.dma_start(
                            kts[:, 0:8, :],
                            kspa_d[r][:, :, lc * 128:(lc + 1) * 128]
                            .rearrange("kc p j -> p kc j"))
                        nc.sync.dma_start(
                            kts[:, 8:16, :],
                            kspb_d[r][:, :, lc * 128:(lc + 1) * 128]
                            .rearrange("kc p j -> p kc j"))
                        this_round = []
                        for e in range(lc // 2, 4):
                            ps_s = pp_s.tile([128, 256], F32, tag="s")
                            for kc in range(NCHK):
                                nc.tensor.matmul(ps_s[:], kts[:, kc, :],
                                                 qT[:, kc, e * 256:(e + 1) * 256],
                                                 start=(kc == 0), stop=(kc == NCHK - 1))
                            dst = aT[:, ABASE[e] + r * 2 * (e + 1) + lc, :]
                            if lc // 2 == e:
                                tmp = bs.tile([128, 256], BF16, tag="exps")
                                nc.scalar.activation(tmp[:], ps_s[:], AF.Exp, scale=SCALE)
                                mt = bm.tile([128, 256], BF16, tag="mask")
                                nc.sync.dma_start(mt[:], mask_d[e, r * 2 + lc % 2])
                                nc.vector.tensor_mul(dst, tmp[:], mt[:])
                            else:
                                nc.scalar.activation(dst, ps_s[:], AF.Exp, scale=SCALE)
                            # (r, lc) == (0, 0) is first for every class; class e
                            # ends at (1, 2e+1) in lc-major-then-r order.
                            this_round.append((e, (r, lc), dst))
                        for e, pos, src2 in pending:
                            nc.tensor.matmul(ps_sums[e][:], ones[:, 0:1], src2,
                                             start=(pos == (0, 0)),
                                             stop=(pos == (1, 2 * e + 1)))
                        pending = this_round
                    for e, pos, src2 in pending:
                        nc.tensor.matmul(ps_sums[e][:], ones[:, 0:1], src2,
                                         start=(pos == (0, 0)),
                                         stop=(pos == (1, 2 * e + 1)))
                    for e in range(4):
                        srow = bs.tile([1, 256], F32, tag="srow")
                        nc.vector.tensor_copy(srow[:], ps_sums[e][:])
                        nc.sync.dma_start(ssp_d[e], srow[:])
                        scol = bs.tile([128, 2], F32, tag="scol")
                        nc.sync.dma_start(scol[:],
                                          ssp_d[e].rearrange("(j p) -> p j", p=128))
                        nc.vector.reciprocal(recip[:, 2 * e:2 * e + 2], scol[:])

                # ---------- C: out^T = v^T-chunks @ attn^T ----------
                with (
                    nc.named_scope("attn_v"),
                    tc.tile_pool(name="cv", bufs=3) as cv,
                    tc.tile_pool(name="pp_o", bufs=2, space=bass.MemorySpace.PSUM) as pp_o,
                ):
                    for kvc in range(NCHK):
                        vts = cv.tile([128, NCHK, 128], BF16, tag="vt")
                        for r in range(2):
                            nc.sync.dma_start(
                                vts[:, r * 8:(r + 1) * 8, :],
                                vsp_d[r][:, :, kvc * 128:(kvc + 1) * 128]
                                .rearrange("lc p j -> p lc j"))
                        for e in range(4):
                            slots = [(r, lc) for r in range(2)
                                     for lc in range(2 * (e + 1))]
                            ps_o = pp_o.tile([128, 256], F32, tag="o")
                            for si, (r, lc) in enumerate(slots):
                                nc.tensor.matmul(
                                    ps_o[:], vts[:, r * 8 + lc, :],
                                    aT[:, ABASE[e] + r * 2 * (e + 1) + lc, :],
                                    start=(si == 0), stop=(si == len(slots) - 1))
                            if e % 2 == 0:
                                nc.scalar.copy(oT[:, kvc, e * 256:(e + 1) * 256],
                                               ps_o[:])
                            else:
                                nc.vector.tensor_copy(oT[:, kvc, e * 256:(e + 1) * 256],
                                                      ps_o[:])

                # ---------- D: y = diag(1/sums) (out @ Wo) + x ----------
                with (
                    nc.named_scope("o_proj"),
                    tc.tile_pool(name="dw", bufs=2) as dw,
                    tc.tile_pool(name="dx", bufs=4) as dx,
                    tc.tile_pool(name="dy", bufs=4) as dy,
                    tc.tile_pool(name="pp_y", bufs=6, space=bass.MemorySpace.PSUM) as pp_y,
                ):
                    for ht in range(4):
                        wo_s = dw.tile([128, NCHK, 512], BF16, tag="wo")
                        nc.sync.dma_start(wo_s[:],
                                          wo_d[:, ht * 512:(ht + 1) * 512]
                                          .rearrange("(kvc p) j -> p kvc j", p=128))
                        for qg in range(2):
                            psy = [pp_y.tile([128, 512], F32, tag="y", name=f"psy{i}")
                                   for i in range(4)]
                            for kvc in range(NCHK):
                                for i in range(4):
                                    qc = qg * 4 + i
                                    nc.tensor.matmul(psy[i][:],
                                                     oT[:, kvc, qc * 128:(qc + 1) * 128],
                                                     wo_s[:, kvc, :],
                                                     start=(kvc == 0),
                                                     stop=(kvc == NCHK - 1))
                            for i in range(4):
                                qc = qg * 4 + i
                                xres = dx.tile([128, 512], F32, tag="xr")
                                nc.sync.dma_start(xres[:],
                                                  x_d[PC[qc] * 128:(PC[qc] + 1) * 128,
                                                      ht * 512:(ht + 1) * 512])
                                ysb = dy.tile([128, 512], F32, tag="y")
                                nc.vector.scalar_tensor_tensor(
                                    ysb[:], psy[i][:], recip[:, qc:qc + 1], xres[:],
                                    OP.mult, OP.add)
                                nc.sync.dma_start(y_d[qc * 128:(qc + 1) * 128,
                                                      ht * 512:(ht + 1) * 512], ysb[:])
                if DBG:
                    nc.sync.dma_start(dbg_k[0], kspa_d[:])
                    nc.sync.dma_start(dbg_k[1], kspb_d[:])
                    nc.sync.dma_start(dbg_v[:], vsp_d[:])
                    nc.sync.dma_start(dbg_s[:], ssp_d[:])
                    for kc in range(NCHK):
                        nc.sync.dma_start(dbg_q[kc], qT[:, kc, :])
            p_bc0.__exit__(None, None, None)
    nc.compile()
    return nc


_NC_CACHE = None


def _get_nc():
    global _NC_CACHE
    if _NC_CACHE is None:
        _NC_CACHE = build()
    return _NC_CACHE


def make_in_maps(x, qkv, o_proj, gamma, beta):
    qkv = np.asarray(qkv)
    wq16 = np.ascontiguousarray(qkv[:, :KEY]).astype(ml_dtypes.bfloat16)
    wk16 = np.ascontiguousarray(qkv[:, KEY:2 * KEY]).astype(ml_dtypes.bfloat16)
    wv16 = np.ascontiguousarray(qkv[:, 2 * KEY:]).astype(ml_dtypes.bfloat16)
    wo16 = np.ascontiguousarray(o_proj).astype(ml_dtypes.bfloat16)
    gamma = np.ascontiguousarray(gamma, dtype=np.float32)
    beta = np.ascontiguousarray(beta, dtype=np.float32)
    in_maps, metas = [], []
    for c in range(8):
        b, h = c // 2, c % 2
        P = perm_chunks(h)
        ti = np.concatenate([np.arange(pc * 128, pc * 128 + 128) for pc in P])
        x_perm = np.ascontiguousarray(x[b][ti], dtype=np.float32)
        # mask[e][2r+j]: k tok-slot (rank r, quad e, j) holds true chunk
        # 4e+2r+j; q col c of class e is true row ti[512e+c].
        mask = np.zeros((4, 4, 128, 256), dtype=ml_dtypes.bfloat16)
        for e in range(4):
            qp = ti[512 * e:512 * e + 256]
            for r in range(2):
                for j in range(2):
                    kp = (4 * e + 2 * r + j) * 128 + np.arange(128)
                    mask[e, 2 * r + j] = (kp[:, None] <= qp[None, :]).astype(
                        ml_dtypes.bfloat16)
        in_maps.append({"x": x_perm, "wq": wq16, "wk": wk16, "wv": wv16,
                        "wo": wo16, "gamma": gamma, "beta": beta, "mask": mask})
        metas.append((b, ti))
    return in_maps, metas


def gather(results, metas, dtype):
    out = np.empty((B, S, H), dtype=dtype)
    qpos = np.concatenate([np.arange(512 * e, 512 * e + 256) for e in range(4)])
    for c, (b, ti) in enumerate(metas):
        out[b][ti[qpos]] = results[c]["y"]
    return out


def kernel(x, qkv, o_proj, gamma, beta, _trace=False):
    x = np.asarray(x, dtype=np.float32)
    nc = _get_nc()
    in_maps, metas = make_in_maps(x, qkv, o_proj, gamma, beta)
    res = run_bass_kernel_spmd(nc, in_maps, core_ids=list(range(8)), trace=_trace)
    out = gather(res.results, metas, np.float32)
    if _trace:
        kernel.last_result = res
    return out


# revision 19
# speedup vs baseline: 2.3506x; 1.0251x over previous
"""Trainium2 Bass kernel: pre-LN single-head causal attention + residual.

Reference computation (B=4, S=2048, H=K=2048, fp32):
    xn = LayerNorm(x) * gamma + beta
    q,k,v = xn @ qkv (split)
    out = causal_softmax(q k^T / sqrt(K)) @ v @ o_proj + x

Sharding: 8 cores = 4 batches x 2 query-halves. Each core gets its batch's
rows PERMUTED so that its query rows sit at fixed positions, arranged in 4
"classes" of 256 query rows whose causal key extent is 512*(e+1) rows --
a load-balanced folded-causal split with identical program shape on all
cores (pure SPMD; per-core behavior comes only from input data: the
permuted x and the causality masks).

On-device pipeline per core (all matmuls bf16, fp32 accumulate in PSUM):
  A0: LayerNorm stats (bn_stats) -> (x-mu)*rstd on ACT -> PE-transpose ->
      evacuate with gamma/beta fold -> x_norm^T bf16 [hid_p, tok_f]
  A1: v and k^T (spilled to DRAM), q^T resident (bf16)
  B:  scores^T = k^T-tiles^T @ q^T per class, exp on ACT (no max-subtract:
      |score*scale| < ~4 so fp32 exp is exact), causal mask multiply,
      denominators via ones-matmul
  C:  out^T = v-tiles^T @ attn^T (v streamed back from DRAM)
  D:  y = diag(1/sums) (out @ o_proj) + x  (normalization folded into the
      PSUM eviction as a per-partition scale; residual added in same op)
"""
import os
import sys

import numpy as np

sys.path.insert(0, "/opt/trn_rl_repo")


def _install_ntff_hook():
    """Register the axon NTFF profile hook bass_utils expects (the image's
    antenv package lacks axon_hooks); degrades to no-op when unavailable."""
    import types
    if "antenv.axon_hooks" in sys.modules:
        return
    try:
        from trn_agent_boot.trn_boot import _ntff_profile_via_ctypes
        hook = _ntff_profile_via_ctypes("/opt/axon/libaxon_pjrt.so")
    except Exception:
        hook = None
    m = types.ModuleType("antenv.axon_hooks")
    m.get_axon_ntff_profile_hook = lambda: hook
    sys.modules["antenv.axon_hooks"] = m


_install_ntff_hook()

import ml_dtypes  # noqa: E402
import concourse.bass as bass  # noqa: E402
import concourse.tile as tile  # noqa: E402
from concourse import bacc, mybir  # noqa: E402
from concourse.bass_utils import run_bass_kernel_spmd  # noqa: E402

F32 = mybir.dt.float32
BF16 = mybir.dt.bfloat16
AF = mybir.ActivationFunctionType
OP = mybir.AluOpType

B, S, H, KEY = 4, 2048, 2048, 2048
NCHK = 16                 # 128-row chunks per sequence
EPS = 1e-5
SCALE = 1.0 / float(np.sqrt(KEY))
ABASE = [0, 4, 12, 24]    # attn^T tile base index per class
ATOT = 40                 # total k-chunk tiles across classes
PC = [0, 1, 4, 5, 8, 9, 12, 13]   # position chunks holding this core's q rows


def perm_chunks(h):
    out = []
    for e in range(4):
        out += [4 * e + 2 * h, 4 * e + 2 * h + 1,
                4 * e + 2 * (1 - h), 4 * e + 2 * (1 - h) + 1]
    return out


def build():
    nc = bacc.Bacc("TRN2", target_bir_lowering=False, debug=False, num_devices=8)

    x_d = nc.dram_tensor("x", [S, H], F32, kind="ExternalInput")
    wq_d = nc.dram_tensor("wq", [H, KEY], BF16, kind="ExternalInput")
    wk_d = nc.dram_tensor("wk", [H, KEY], BF16, kind="ExternalInput")
    wv_d = nc.dram_tensor("wv", [H, KEY], BF16, kind="ExternalInput")
    wo_d = nc.dram_tensor("wo", [KEY, H], BF16, kind="ExternalInput")
    gamma_d = nc.dram_tensor("gamma", [H], F32, kind="ExternalInput")
    beta_d = nc.dram_tensor("beta", [H], F32, kind="ExternalInput")
    mask_d = nc.dram_tensor("mask", [4, 4, 128, 256], BF16, kind="ExternalInput")
    y_d = nc.dram_tensor("y", [1024, H], F32, kind="ExternalOutput")
    DBG = bool(os.environ.get("K_DEBUG"))
    if DBG:
        dbg_k = nc.dram_tensor("dbg_k", [2, 2, 8, 128, 1024], BF16,
                               kind="ExternalOutput")
        dbg_v = nc.dram_tensor("dbg_v", [2, 8, 128, KEY], BF16,
                               kind="ExternalOutput")
        dbg_s = nc.dram_tensor("dbg_s", [4, 256], F32, kind="ExternalOutput")
        dbg_q = nc.dram_tensor("dbg_q", [NCHK, 128, 1024], BF16, kind="ExternalOutput")
    ssp_d = nc.dram_tensor("ssp", [4, 256], F32, kind="Internal")
    vsl_d = nc.dram_tensor("vsl", [8, 128, KEY], BF16, kind="Internal")
    vsp_d = nc.dram_tensor("vsp", [2, 8, 128, KEY], BF16, kind="Internal")
    ksla_d = nc.dram_tensor("ksla", [8, 128, 1024], BF16, kind="Internal")
    kslb_d = nc.dram_tensor("kslb", [8, 128, 1024], BF16, kind="Internal")
    kspa_d = nc.dram_tensor("kspa", [2, 8, 128, 1024], BF16, kind="Internal")
    kspb_d = nc.dram_tensor("kspb", [2, 8, 128, 1024], BF16, kind="Internal")
    GROUPS = [[2 * p, 2 * p + 1] for p in range(4)]

    import ml_dtypes as _mld
    ident = nc.inline_tensor(np.eye(128).astype(_mld.bfloat16), name="ident")

    with tile.TileContext(nc) as tc:
        with (
            tc.tile_pool(name="small", bufs=1) as small,
            tc.tile_pool(name="p_main", bufs=1) as p_main,
        ):
            xnT = p_main.tile([128, NCHK, S], BF16)   # x_norm^T  [hid, tok]
            recip = small.tile([128, 8], F32)         # 1/sums per q-chunk
            gcol = small.tile([128, NCHK], F32)       # gamma, [p, hc]
            bcol = small.tile([128, NCHK], F32)       # beta
            ones = small.tile([128, 1], BF16)
            id16_sb = small.tile([128, 128], BF16)

            nc.sync.dma_start(gcol[:], gamma_d[:].rearrange("(c p) -> p c", p=128))
            nc.sync.dma_start(bcol[:], beta_d[:].rearrange("(c p) -> p c", p=128))
            nc.sync.dma_start(id16_sb[:], ident[:])
            nc.vector.memset(ones[:], 1.0)

            # ---------- A0: LN + transpose, interleaved with v-half ----------
            with (
                nc.named_scope("ln_transpose"),
                tc.tile_pool(name="a0x", bufs=3) as a0x,
                tc.tile_pool(name="a0xp", bufs=8) as a0xp,
                tc.tile_pool(name="a0s", bufs=4) as a0s,
                tc.tile_pool(name="wv", bufs=1) as wvp,
                tc.tile_pool(name="vst", bufs=2) as vst,
                tc.tile_pool(name="pp_tr", bufs=2, space=bass.MemorySpace.PSUM) as pp_tr,
                tc.tile_pool(name="pp_v", bufs=4, space=bass.MemorySpace.PSUM) as pp_v,
            ):
                wv_sb = wvp.tile([128, NCHK, KEY], BF16)
                for hc in range(NCHK):
                    nc.sync.dma_start(wv_sb[:, hc, :],
                                      wv_d[hc * 128:(hc + 1) * 128, :])
                for tg in range(4):
                    xps = []
                    for i in range(4):
                        tci = tg * 4 + i
                        x_t = a0x.tile([128, H], F32, tag="x")
                        nc.sync.dma_start(x_t[:], x_d[tci * 128:(tci + 1) * 128, :])
                        st = a0s.tile([128, 4, 6], F32, tag="st")
                        for j in range(4):
                            nc.vector.bn_stats(st[:, j, :], x_t[:, j * 512:(j + 1) * 512])
                        ag = a0s.tile([128, 2], F32, tag="ag")
                        nc.vector.bn_aggr(ag[:], st[:])
                        veps = a0s.tile([128, 1], F32, tag="veps")
                        nc.vector.tensor_scalar_add(veps[:], ag[:, 1:2], EPS)
                        sq = a0s.tile([128, 1], F32, tag="sq")
                        nc.scalar.sqrt(sq[:], veps[:])
                        rstd = a0s.tile([128, 1], F32, tag="rstd")
                        nc.vector.reciprocal(rstd[:], sq[:])
                        nmr = a0s.tile([128, 1], F32, tag="nmr")
                        nc.vector.tensor_scalar(nmr[:], ag[:, 0:1], rstd[:], -1.0,
                                                OP.mult, OP.mult)
                        xp = a0xp.tile([128, H], BF16, tag="xp")
                        nc.vector.tensor_scalar(xp[:], x_t[:], rstd[:], nmr[:],
                                                OP.mult, OP.add)
                        xps.append(xp)
                    for hc in range(NCHK):
                        ps = pp_tr.tile([128, 512], BF16, tag="tr")
                        for i in range(4):
                            nc.tensor.transpose(ps[:, i * 128:(i + 1) * 128],
                                                xps[i][:, hc * 128:(hc + 1) * 128],
                                                id16_sb[:])
                        dst = xnT[:, hc, tg * 512:(tg + 1) * 512]
                        nc.scalar.activation(dst, ps[:], AF.Identity,
                                             bias=bcol[:, hc:hc + 1],
                                             scale=gcol[:, hc:hc + 1])
                    for i in range(2):
                        tci = tg * 4 + i          # own q-position chunks 4e, 4e+1
                        vs = vst.tile([128, KEY], BF16, tag="vs")
                        for kvt in range(4):
                            ps = pp_v.tile([128, 512], F32, tag="v")
                            for hc in range(NCHK):
                                nc.tensor.matmul(ps[:],
                                                 xnT[:, hc, tci * 128:(tci + 1) * 128],
                                                 wv_sb[:, hc, kvt * 512:(kvt + 1) * 512],
                                                 start=(hc == 0), stop=(hc == NCHK - 1))
                            nc.scalar.copy(vs[:, kvt * 512:(kvt + 1) * 512], ps[:])
                        nc.scalar.dma_start(vsl_d[tg * 2 + i][:], vs[:])
            nc.gpsimd.collective_compute(
                "AllGather", OP.bypass, replica_groups=GROUPS,
                ins=[vsl_d.ap().opt()], outs=[vsp_d.ap().opt()])

            # ---------- pools for q^T, attn^T, out^T (A1b..D) ----------
            p_bc0 = tc.tile_pool(name="p_bc", bufs=1)
            p_bc = p_bc0.__enter__()
            qT = p_bc.tile([128, NCHK, 1024], BF16)   # q^T [key, class-packed q]
            aT = p_bc.tile([128, ATOT, 256], BF16)    # attn^T tiles
            oT = p_bc.tile([128, NCHK, 1024], BF16)   # out^T [kv, q]

            # ---------- A1b: q^T (resident), k^T (spilled) ----------
            with (
                nc.named_scope("qk_proj"),
                tc.tile_pool(name="wqk", bufs=6) as wqk,
                tc.tile_pool(name="kst", bufs=2) as kst,
                tc.tile_pool(name="pp_q", bufs=3, space=bass.MemorySpace.PSUM) as pp_q,
                tc.tile_pool(name="pp_k", bufs=5, space=bass.MemorySpace.PSUM) as pp_k,
            ):
                wk_tiles = []
                for kc in range(NCHK):
                    wk_s = wqk.tile([128, NCHK, 128], BF16, tag="wk",
                                    name=f"wk_s{kc}")
                    nc.sync.dma_start(
                        wk_s[:], wk_d[:, kc * 128:(kc + 1) * 128]
                        .rearrange("(hc p) j -> p hc j", p=128))
                    wk_tiles.append(wk_s)
                for kc in range(NCHK):
                    wk_s = wk_tiles[kc]
                    psk_a = pp_k.tile([128, 512], F32, tag="k")
                    psk_b = pp_k.tile([128, 512], F32, tag="k")
                    for hc in range(NCHK):
                        xr3 = xnT[:, hc, :].rearrange("p (g r) -> p g r", r=256)
                        st, sp = (hc == 0), (hc == NCHK - 1)
                        nc.tensor.matmul(psk_a[:], wk_s[:, hc, :], xr3[:, 0:3:2, :],
                                         start=st, stop=sp)
                        nc.tensor.matmul(psk_b[:], wk_s[:, hc, :], xr3[:, 4:7:2, :],
                                         start=st, stop=sp)
                    ks = kst.tile([128, 1024], BF16, tag="ks")
                    nc.scalar.copy(ks[:, 0:512], psk_a[:])
                    nc.scalar.copy(ks[:, 512:1024], psk_b[:])
                    ksl = ksla_d if kc < 8 else kslb_d
                    nc.scalar.dma_start(ksl[kc % 8][:], ks[:])
                    if kc == 7:
                        nc.gpsimd.collective_compute(
                            "AllGather", OP.bypass, replica_groups=GROUPS,
                            ins=[ksla_d.ap().opt()], outs=[kspa_d.ap().opt()])
                nc.gpsimd.collective_compute(
                    "AllGather", OP.bypass, replica_groups=GROUPS,
                    ins=[kslb_d.ap().opt()], outs=[kspb_d.ap().opt()])
                wq_tiles = []
                for kc in range(NCHK):
                    wq_s = wqk.tile([128, NCHK, 128], BF16, tag="wq",
                                    name=f"wq_s{kc}")
                    nc.sync.dma_start(
                        wq_s[:], wq_d[:, kc * 128:(kc + 1) * 128]
                        .rearrange("(hc p) j -> p hc j", p=128))
                    wq_tiles.append(wq_s)
                for kc in range(NCHK):
                    wq_s = wq_tiles[kc]
                    psq_a = pp_q.tile([128, 512], F32, tag="q")
                    psq_b = pp_q.tile([128, 512], F32, tag="q")
                    for hc in range(NCHK):
                        xr3 = xnT[:, hc, :].rearrange("p (g r) -> p g r", r=256)
                        st, sp = (hc == 0), (hc == NCHK - 1)
                        nc.tensor.matmul(psq_a[:], wq_s[:, hc, :], xr3[:, 0:3:2, :],
                                         start=st, stop=sp)
                        nc.tensor.matmul(psq_b[:], wq_s[:, hc, :], xr3[:, 4:7:2, :],
                                         start=st, stop=sp)
                    nc.vector.tensor_copy(qT[:, kc, 0:512], psq_a[:])
                    nc.vector.tensor_copy(qT[:, kc, 512:1024], psq_b[:])

            # ---------- B..D ----------
            if True:
                # ---------- B: scores^T -> exp -> mask -> attn^T, denominators ----
                with (
                    nc.named_scope("scores"),
                    tc.tile_pool(name="bk", bufs=3) as bk,
                    tc.tile_pool(name="bm", bufs=4) as bm,
                    tc.tile_pool(name="bs", bufs=4) as bs,
                    tc.tile_pool(name="pp_s", bufs=3, space=bass.MemorySpace.PSUM) as pp_s,
                    tc.tile_pool(name="pp_sum", bufs=4,
                                 space=bass.MemorySpace.PSUM) as pp_sum,
                ):
                    # tok-slot s = r*8 + lc: rank r's local tok chunk lc
                    # (true chunk 4*(lc//2) + 2r + lc%2); class e consumes
                    # slots with lc < 2(e+1).
                    ps_sums = [pp_sum.tile([1, 256], F32, tag="sum", name=f"psum{e}")
                               for e in range(4)]
                    pending = []
                    SLOT_ORDER = [(r, lc) for lc in range(8) for r in range(2)]
                    for r, lc in SLOT_ORDER:
                        kts = bk.tile([128, NCHK, 128], BF16, tag="kt")
                        nc.sync.dma_start(
                            kts[:, 0:8, :],
                            kspa_d[r][:, :, lc * 128:(lc + 1) * 128]
                            .rearrange("kc p j -> p kc j"))
                        nc.sync.dma_start(
                            kts[:, 8:16, :],
                            kspb_d[r][:, :, lc * 128:(lc + 1) * 128]
                            .rearrange("kc p j -> p kc j"))
                        this_round = []
                        for e in range(lc // 2, 4):
                            ps_s = pp_s.tile([128, 256], F32, tag="s")
                            for kc in range(NCHK):
                                nc.tensor.matmul(ps_s[:], kts[:, kc, :],
                                                 qT[:, kc, e * 256:(e + 1) * 256],
                                                 start=(kc == 0), stop=(kc == NCHK - 1))
                            dst = aT[:, ABASE[e] + r * 2 * (e + 1) + lc, :]
                            if lc // 2 == e:
                                tmp = bs.tile([128, 256], BF16, tag="exps")
                                nc.scalar.activation(tmp[:], ps_s[:], AF.Exp, scale=SCALE)
                                mt = bm.tile([128, 256], BF16, tag="mask")
                                nc.sync.dma_start(mt[:], mask_d[e, r * 2 + lc % 2])
                                nc.vector.tensor_mul(dst, tmp[:], mt[:])
                            else:
                                nc.scalar.activation(dst, ps_s[:], AF.Exp, scale=SCALE)
                            # (r, lc) == (0, 0) is first for every class; class e
                            # ends at (1, 2e+1) in lc-major-then-r order.
                            this_round.append((e, (r, lc), dst))
                        for e, pos, src2 in pending:
                            nc.tensor.matmul(ps_sums[e][:], ones[:, 0:1], src2,
                                             start=(pos == (0, 0)),
                                             stop=(pos == (1, 2 * e + 1)))
                        pending = this_round
                    for e, pos, src2 in pending:
                        nc.tensor.matmul(ps_sums[e][:], ones[:, 0:1], src2,
                                         start=(pos == (0, 0)),
                                         stop=(pos == (1, 2 * e + 1)))
                    for e in range(4):
                        srow = bs.tile([1, 256], F32, tag="srow", name=f"srow{e}")
                        nc.scalar.copy(srow[:], ps_sums[e][:])
                        nc.scalar.dma_start(ssp_d[e], srow[:])
                    for e in range(4):
                        scol = bs.tile([128, 2], F32, tag="scol", name=f"scol{e}")
                        nc.sync.dma_start(scol[:],
                                          ssp_d[e].rearrange("(j p) -> p j", p=128))
                        nc.vector.reciprocal(recip[:, 2 * e:2 * e + 2], scol[:])

                # ---------- C: out^T = v^T-chunks @ attn^T ----------
                with (
                    nc.named_scope("attn_v"),
                    tc.tile_pool(name="cv", bufs=3) as cv,
                    tc.tile_pool(name="pp_o", bufs=2, space=bass.MemorySpace.PSUM) as pp_o,
                ):
                    for kvc in range(NCHK):
                        vts = cv.tile([128, NCHK, 128], BF16, tag="vt")
                        for r in range(2):
                            nc.sync.dma_start(
                                vts[:, r * 8:(r + 1) * 8, :],
                                vsp_d[r][:, :, kvc * 128:(kvc + 1) * 128]
                                .rearrange("lc p j -> p lc j"))
                        for e in range(4):
                            slots = [(r, lc) for r in range(2)
                                     for lc in range(2 * (e + 1))]
                            ps_o = pp_o.tile([128, 256], F32, tag="o")
                            for si, (r, lc) in enumerate(slots):
                                nc.tensor.matmul(
                                    ps_o[:], vts[:, r * 8 + lc, :],
                                    aT[:, ABASE[e] + r * 2 * (e + 1) + lc, :],
                                    start=(si == 0), stop=(si == len(slots) - 1))
                            if e % 2 == 0:
                                nc.scalar.copy(oT[:, kvc, e * 256:(e + 1) * 256],
                                               ps_o[:])
                            else:
                                nc.vector.tensor_copy(oT[:, kvc, e * 256:(e + 1) * 256],
                                                      ps_o[:])

                # ---------- D: y = diag(1/sums) (out @ Wo) + x ----------
                with (
                    nc.named_scope("o_proj"),
                    tc.tile_pool(name="dw", bufs=2) as dw,
                    tc.tile_pool(name="dx", bufs=4) as dx,
                    tc.tile_pool(name="dy", bufs=4) as dy,
                    tc.tile_pool(name="pp_y", bufs=6, space=bass.MemorySpace.PSUM) as pp_y,
                ):
                    for ht in range(4):
                        wo_s = dw.tile([128, NCHK, 512], BF16, tag="wo")
                        nc.sync.dma_start(wo_s[:],
                                          wo_d[:, ht * 512:(ht + 1) * 512]
                                          .rearrange("(kvc p) j -> p kvc j", p=128))
                        for qg in range(2):
                            psy = [pp_y.tile([128, 512], F32, tag="y", name=f"psy{i}")
                                   for i in range(4)]
                            for kvc in range(NCHK):
                                for i in range(4):
                                    qc = qg * 4 + i
                                    nc.tensor.matmul(psy[i][:],
                                                     oT[:, kvc, qc * 128:(qc + 1) * 128],
                                                     wo_s[:, kvc, :],
                                                     start=(kvc == 0),
                                                     stop=(kvc == NCHK - 1))
                            for i in range(4):
                                qc = qg * 4 + i
                                xres = dx.tile([128, 512], F32, tag="xr")
                                nc.sync.dma_start(xres[:],
                                                  x_d[PC[qc] * 128:(PC[qc] + 1) * 128,
                                                      ht * 512:(ht + 1) * 512])
                                ysb = dy.tile([128, 512], F32, tag="y")
                                nc.vector.scalar_tensor_tensor(
                                    ysb[:], psy[i][:], recip[:, qc:qc + 1], xres[:],
                                    OP.mult, OP.add)
                                nc.scalar.dma_start(y_d[qc * 128:(qc + 1) * 128,
                                                        ht * 512:(ht + 1) * 512],
                                                    ysb[:])
                if DBG:
                    nc.sync.dma_start(dbg_k[0], kspa_d[:])
                    nc.sync.dma_start(dbg_k[1], kspb_d[:])
                    nc.sync.dma_start(dbg_v[:], vsp_d[:])
                    nc.sync.dma_start(dbg_s[:], ssp_d[:])
                    for kc in range(NCHK):
                        nc.sync.dma_start(dbg_q[kc], qT[:, kc, :])
            p_bc0.__exit__(None, None, None)
    nc.compile()
    return nc


_NC_CACHE = None


def _get_nc():
    global _NC_CACHE
    if _NC_CACHE is None:
        _NC_CACHE = build()
    return _NC_CACHE


def make_in_maps(x, qkv, o_proj, gamma, beta):
    qkv = np.asarray(qkv)
    wq16 = np.ascontiguousarray(qkv[:, :KEY]).astype(ml_dtypes.bfloat16)
    wk16 = np.ascontiguousarray(qkv[:, KEY:2 * KEY]).astype(ml_dtypes.bfloat16)
    wv16 = np.ascontiguousarray(qkv[:, 2 * KEY:]).astype(ml_dtypes.bfloat16)
    wo16 = np.ascontiguousarray(o_proj).astype(ml_dtypes.bfloat16)
    gamma = np.ascontiguousarray(gamma, dtype=np.float32)
    beta = np.ascontiguousarray(beta, dtype=np.float32)
    in_maps, metas = [], []
    for c in range(8):
        b, h = c // 2, c % 2
        P = perm_chunks(h)
        ti = np.concatenate([np.arange(pc * 128, pc * 128 + 128) for pc in P])
        x_perm = np.ascontiguousarray(x[b][ti], dtype=np.float32)
        # mask[e][2r+j]: k tok-slot (rank r, quad e, j) holds true chunk
        # 4e+2r+j; q col c of class e is true row ti[512e+c].
        mask = np.zeros((4, 4, 128, 256), dtype=ml_dtypes.bfloat16)
        for e in range(4):
            qp = ti[512 * e:512 * e + 256]
            for r in range(2):
                for j in range(2):
                    kp = (4 * e + 2 * r + j) * 128 + np.arange(128)
                    mask[e, 2 * r + j] = (kp[:, None] <= qp[None, :]).astype(
                        ml_dtypes.bfloat16)
        in_maps.append({"x": x_perm, "wq": wq16, "wk": wk16, "wv": wv16,
                        "wo": wo16, "gamma": gamma, "beta": beta, "mask": mask})
        metas.append((b, ti))
    return in_maps, metas


def gather(results, metas, dtype):
    out = np.empty((B, S, H), dtype=dtype)
    qpos = np.concatenate([np.arange(512 * e, 512 * e + 256) for e in range(4)])
    for c, (b, ti) in enumerate(metas):
        out[b][ti[qpos]] = results[c]["y"]
    return out


def kernel(x, qkv, o_proj, gamma, beta, _trace=False):
    x = np.asarray(x, dtype=np.float32)
    nc = _get_nc()
    in_maps, metas = make_in_maps(x, qkv, o_proj, gamma, beta)
    res = run_bass_kernel_spmd(nc, in_maps, core_ids=list(range(8)), trace=_trace)
    out = gather(res.results, metas, np.float32)
    if _trace:
        kernel.last_result = res
    return out


# revision 22
# speedup vs baseline: 2.3577x; 1.0030x over previous
"""Trainium2 Bass kernel: pre-LN single-head causal attention + residual.

Reference computation (B=4, S=2048, H=K=2048, fp32):
    xn = LayerNorm(x) * gamma + beta
    q,k,v = xn @ qkv (split)
    out = causal_softmax(q k^T / sqrt(K)) @ v @ o_proj + x

Sharding: 8 cores = 4 batches x 2 query-halves. Each core gets its batch's
rows PERMUTED so that its query rows sit at fixed positions, arranged in 4
"classes" of 256 query rows whose causal key extent is 512*(e+1) rows --
a load-balanced folded-causal split with identical program shape on all
cores (pure SPMD; per-core behavior comes only from input data: the
permuted x and the causality masks).

On-device pipeline per core (all matmuls bf16, fp32 accumulate in PSUM):
  A0: LayerNorm stats (bn_stats) -> (x-mu)*rstd on ACT -> PE-transpose ->
      evacuate with gamma/beta fold -> x_norm^T bf16 [hid_p, tok_f]
  A1: v and k^T (spilled to DRAM), q^T resident (bf16)
  B:  scores^T = k^T-tiles^T @ q^T per class, exp on ACT (no max-subtract:
      |score*scale| < ~4 so fp32 exp is exact), causal mask multiply,
      denominators via ones-matmul
  C:  out^T = v-tiles^T @ attn^T (v streamed back from DRAM)
  D:  y = diag(1/sums) (out @ o_proj) + x  (normalization folded into the
      PSUM eviction as a per-partition scale; residual added in same op)
"""
import os
import sys

import numpy as np

sys.path.insert(0, "/opt/trn_rl_repo")


def _install_ntff_hook():
    """Register the axon NTFF profile hook bass_utils expects (the image's
    antenv package lacks axon_hooks); degrades to no-op when unavailable."""
    import types
    if "antenv.axon_hooks" in sys.modules:
        return
    try:
        from trn_agent_boot.trn_boot import _ntff_profile_via_ctypes
        hook = _ntff_profile_via_ctypes("/opt/axon/libaxon_pjrt.so")
    except Exception:
        hook = None
    m = types.ModuleType("antenv.axon_hooks")
    m.get_axon_ntff_profile_hook = lambda: hook
    sys.modules["antenv.axon_hooks"] = m


_install_ntff_hook()

import ml_dtypes  # noqa: E402
import concourse.bass as bass  # noqa: E402
import concourse.tile as tile  # noqa: E402
from concourse import bacc, mybir  # noqa: E402
from concourse.bass_utils import run_bass_kernel_spmd  # noqa: E402

F32 = mybir.dt.float32
BF16 = mybir.dt.bfloat16
AF = mybir.ActivationFunctionType
OP = mybir.AluOpType

B, S, H, KEY = 4, 2048, 2048, 2048
NCHK = 16                 # 128-row chunks per sequence
EPS = 1e-5
SCALE = 1.0 / float(np.sqrt(KEY))
ABASE = [0, 4, 12, 24]    # attn^T tile base index per class
ATOT = 40                 # total k-chunk tiles across classes
PC = [0, 1, 4, 5, 8, 9, 12, 13]   # position chunks holding this core's q rows


def perm_chunks(h):
    out = []
    for e in range(4):
        out += [4 * e + 2 * h, 4 * e + 2 * h + 1,
                4 * e + 2 * (1 - h), 4 * e + 2 * (1 - h) + 1]
    return out


def build():
    nc = bacc.Bacc("TRN2", target_bir_lowering=False, debug=False, num_devices=8)

    x_d = nc.dram_tensor("x", [S, H], F32, kind="ExternalInput")
    wq_d = nc.dram_tensor("wq", [H, KEY], BF16, kind="ExternalInput")
    wk_d = nc.dram_tensor("wk", [H, KEY], BF16, kind="ExternalInput")
    wv_d = nc.dram_tensor("wv", [H, KEY], BF16, kind="ExternalInput")
    wo_d = nc.dram_tensor("wo", [KEY, H], BF16, kind="ExternalInput")
    gamma_d = nc.dram_tensor("gamma", [H], F32, kind="ExternalInput")
    beta_d = nc.dram_tensor("beta", [H], F32, kind="ExternalInput")
    mask_d = nc.dram_tensor("mask", [4, 4, 128, 256], BF16, kind="ExternalInput")
    y_d = nc.dram_tensor("y", [1024, H], F32, kind="ExternalOutput")
    DBG = bool(os.environ.get("K_DEBUG"))
    if DBG:
        dbg_k = nc.dram_tensor("dbg_k", [2, 2, 8, 128, 1024], BF16,
                               kind="ExternalOutput")
        dbg_v = nc.dram_tensor("dbg_v", [2, 8, 128, KEY], BF16,
                               kind="ExternalOutput")
        dbg_s = nc.dram_tensor("dbg_s", [4, 256], F32, kind="ExternalOutput")
        dbg_q = nc.dram_tensor("dbg_q", [NCHK, 128, 1024], BF16, kind="ExternalOutput")
    ssp_d = nc.dram_tensor("ssp", [4, 256], F32, kind="Internal")
    vsl_d = nc.dram_tensor("vsl", [8, 128, KEY], BF16, kind="Internal")
    vsp_d = nc.dram_tensor("vsp", [2, 8, 128, KEY], BF16, kind="Internal")
    ksla_d = nc.dram_tensor("ksla", [NCHK, 128, 512], BF16, kind="Internal")
    kslb_d = nc.dram_tensor("kslb", [NCHK, 128, 512], BF16, kind="Internal")
    kspa_d = nc.dram_tensor("kspa", [2, NCHK, 128, 512], BF16, kind="Internal")
    kspb_d = nc.dram_tensor("kspb", [2, NCHK, 128, 512], BF16, kind="Internal")
    GROUPS = [[2 * p, 2 * p + 1] for p in range(4)]

    import ml_dtypes as _mld
    ident = nc.inline_tensor(np.eye(128).astype(_mld.bfloat16), name="ident")

    with tile.TileContext(nc) as tc:
        with (
            tc.tile_pool(name="small", bufs=1) as small,
            tc.tile_pool(name="p_main", bufs=1) as p_main,
        ):
            xnT = p_main.tile([128, NCHK, S], BF16)   # x_norm^T  [hid, tok]
            recip = small.tile([128, 8], F32)         # 1/sums per q-chunk
            gcol = small.tile([128, NCHK], F32)       # gamma, [p, hc]
            bcol = small.tile([128, NCHK], F32)       # beta
            ones = small.tile([128, 1], BF16)
            id16_sb = small.tile([128, 128], BF16)

            nc.sync.dma_start(gcol[:], gamma_d[:].rearrange("(c p) -> p c", p=128))
            nc.sync.dma_start(bcol[:], beta_d[:].rearrange("(c p) -> p c", p=128))
            nc.sync.dma_start(id16_sb[:], ident[:])
            nc.vector.memset(ones[:], 1.0)

            # ---------- A0: LN + transpose, interleaved with v-half ----------
            with (
                nc.named_scope("ln_transpose"),
                tc.tile_pool(name="a0x", bufs=2) as a0x,
                tc.tile_pool(name="a0xp", bufs=8) as a0xp,
                tc.tile_pool(name="a0s", bufs=4) as a0s,
                tc.tile_pool(name="wv", bufs=1) as wvp,
                tc.tile_pool(name="vst", bufs=2) as vst,
                tc.tile_pool(name="wqk0", bufs=5) as wqk0,
                tc.tile_pool(name="kst", bufs=2) as kst,
                tc.tile_pool(name="pp_tr", bufs=2, space=bass.MemorySpace.PSUM) as pp_tr,
                tc.tile_pool(name="pp_v", bufs=4, space=bass.MemorySpace.PSUM) as pp_v,
                tc.tile_pool(name="pp_k", bufs=2, space=bass.MemorySpace.PSUM) as pp_k,
            ):
                wv_sb = wvp.tile([128, NCHK, KEY], BF16)

                def k_pass(half):
                    """k^T for tok quads (2*half, 2*half+1): needs xnT of
                    token groups 0..2*half+1; spills then gathers."""
                    gstart = 4 * half
                    ksl = [ksla_d, kslb_d][half]
                    gout = [kspa_d, kspb_d][half]
                    wk_tiles = []
                    for kc in range(NCHK):
                        wk_s = wqk0.tile([128, NCHK, 128], BF16, tag="wk",
                                         name=f"wk_s{half}_{kc}")
                        nc.sync.dma_start(
                            wk_s[:], wk_d[:, kc * 128:(kc + 1) * 128]
                            .rearrange("(hc p) j -> p hc j", p=128))
                        wk_tiles.append(wk_s)
                    for kc in range(NCHK):
                        psk = pp_k.tile([128, 512], F32, tag="k")
                        for hc in range(NCHK):
                            xr3 = xnT[:, hc, :].rearrange("p (g r) -> p g r", r=256)
                            nc.tensor.matmul(psk[:], wk_tiles[kc][:, hc, :],
                                             xr3[:, gstart:gstart + 3:2, :],
                                             start=(hc == 0), stop=(hc == NCHK - 1))
                        ks = kst.tile([128, 512], BF16, tag="ks")
                        nc.scalar.copy(ks[:], psk[:])
                        nc.scalar.dma_start(ksl[kc][:], ks[:])
                    nc.gpsimd.collective_compute(
                        "AllGather", OP.bypass, replica_groups=GROUPS,
                        ins=[ksl.ap().opt()], outs=[gout.ap().opt()])

                for hc in range(NCHK):
                    nc.scalar.dma_start(wv_sb[:, hc, :],
                                        wv_d[hc * 128:(hc + 1) * 128, :])
                for tg in range(4):
                    xps = []
                    for i in range(4):
                        tci = tg * 4 + i
                        x_t = a0x.tile([128, H], F32, tag="x")
                        nc.sync.dma_start(x_t[:], x_d[tci * 128:(tci + 1) * 128, :])
                        st = a0s.tile([128, 4, 6], F32, tag="st")
                        for j in range(4):
                            nc.vector.bn_stats(st[:, j, :], x_t[:, j * 512:(j + 1) * 512])
                        ag = a0s.tile([128, 2], F32, tag="ag")
                        nc.vector.bn_aggr(ag[:], st[:])
                        veps = a0s.tile([128, 1], F32, tag="veps")
                        nc.vector.tensor_scalar_add(veps[:], ag[:, 1:2], EPS)
                        sq = a0s.tile([128, 1], F32, tag="sq")
                        nc.scalar.sqrt(sq[:], veps[:])
                        rstd = a0s.tile([128, 1], F32, tag="rstd")
                        nc.vector.reciprocal(rstd[:], sq[:])
                        nmr = a0s.tile([128, 1], F32, tag="nmr")
                        nc.vector.tensor_scalar(nmr[:], ag[:, 0:1], rstd[:], -1.0,
                                                OP.mult, OP.mult)
                        xp = a0xp.tile([128, H], BF16, tag="xp")
                        nc.vector.tensor_scalar(xp[:], x_t[:], rstd[:], nmr[:],
                                                OP.mult, OP.add)
                        xps.append(xp)
                    for hc in range(NCHK):
                        ps = pp_tr.tile([128, 512], BF16, tag="tr")
                        for i in range(4):
                            nc.tensor.transpose(ps[:, i * 128:(i + 1) * 128],
                                                xps[i][:, hc * 128:(hc + 1) * 128],
                                                id16_sb[:])
                        dst = xnT[:, hc, tg * 512:(tg + 1) * 512]
                        nc.scalar.activation(dst, ps[:], AF.Identity,
                                             bias=bcol[:, hc:hc + 1],
                                             scale=gcol[:, hc:hc + 1])
                    for i in range(2):
                        tci = tg * 4 + i          # own q-position chunks 4e, 4e+1
                        vs = vst.tile([128, KEY], BF16, tag="vs")
                        for kvt in range(4):
                            ps = pp_v.tile([128, 512], F32, tag="v")
                            for hc in range(NCHK):
                                nc.tensor.matmul(ps[:],
                                                 xnT[:, hc, tci * 128:(tci + 1) * 128],
                                                 wv_sb[:, hc, kvt * 512:(kvt + 1) * 512],
                                                 start=(hc == 0), stop=(hc == NCHK - 1))
                            nc.scalar.copy(vs[:, kvt * 512:(kvt + 1) * 512], ps[:])
                        nc.scalar.dma_start(vsl_d[tg * 2 + i][:], vs[:])
                    if tg == 1 or tg == 3:
                        k_pass(tg // 2)
            nc.gpsimd.collective_compute(
                "AllGather", OP.bypass, replica_groups=GROUPS,
                ins=[vsl_d.ap().opt()], outs=[vsp_d.ap().opt()])

            # ---------- pools for q^T, attn^T, out^T (A1b..D) ----------
            p_bc0 = tc.tile_pool(name="p_bc", bufs=1)
            p_bc = p_bc0.__enter__()
            qT = p_bc.tile([128, NCHK, 1024], BF16)   # q^T [key, class-packed q]
            aT = p_bc.tile([128, ATOT, 256], BF16)    # attn^T tiles
            oT = p_bc.tile([128, NCHK, 1024], BF16)   # out^T [kv, q]

            # ---------- A1b: q^T (resident), k^T (spilled) ----------
            with (
                nc.named_scope("qk_proj"),
                tc.tile_pool(name="wqk", bufs=6) as wqk,
                tc.tile_pool(name="pp_q", bufs=4, space=bass.MemorySpace.PSUM) as pp_q,
            ):
                wq_tiles = []
                for kc in range(NCHK):
                    wq_s = wqk.tile([128, NCHK, 128], BF16, tag="wq",
                                    name=f"wq_s{kc}")
                    nc.sync.dma_start(
                        wq_s[:], wq_d[:, kc * 128:(kc + 1) * 128]
                        .rearrange("(hc p) j -> p hc j", p=128))
                    wq_tiles.append(wq_s)
                for kc in range(NCHK):
                    wq_s = wq_tiles[kc]
                    psq_a = pp_q.tile([128, 512], F32, tag="q")
                    psq_b = pp_q.tile([128, 512], F32, tag="q")
                    for hc in range(NCHK):
                        xr3 = xnT[:, hc, :].rearrange("p (g r) -> p g r", r=256)
                        st, sp = (hc == 0), (hc == NCHK - 1)
                        nc.tensor.matmul(psq_a[:], wq_s[:, hc, :], xr3[:, 0:3:2, :],
                                         start=st, stop=sp)
                        nc.tensor.matmul(psq_b[:], wq_s[:, hc, :], xr3[:, 4:7:2, :],
                                         start=st, stop=sp)
                    nc.vector.tensor_copy(qT[:, kc, 0:512], psq_a[:])
                    nc.vector.tensor_copy(qT[:, kc, 512:1024], psq_b[:])

            # ---------- B..D ----------
            if True:
                # ---------- B: scores^T -> exp -> mask -> attn^T, denominators ----
                with (
                    nc.named_scope("scores"),
                    tc.tile_pool(name="bk", bufs=3) as bk,
                    tc.tile_pool(name="bm", bufs=4) as bm,
                    tc.tile_pool(name="bs", bufs=4) as bs,
                    tc.tile_pool(name="pp_s", bufs=3, space=bass.MemorySpace.PSUM) as pp_s,
                    tc.tile_pool(name="pp_sum", bufs=4,
                                 space=bass.MemorySpace.PSUM) as pp_sum,
                ):
                    # tok-slot s = r*8 + lc: rank r's local tok chunk lc
                    # (true chunk 4*(lc//2) + 2r + lc%2); class e consumes
                    # slots with lc < 2(e+1).
                    ps_sums = [pp_sum.tile([1, 256], F32, tag="sum", name=f"psum{e}")
                               for e in range(4)]
                    pending = []
                    SLOT_ORDER = [(r, lc) for lc in range(8) for r in range(2)]
                    for r, lc in SLOT_ORDER:
                        src = kspa_d if lc < 4 else kspb_d
                        lcc = lc % 4
                        kts = bk.tile([128, NCHK, 128], BF16, tag="kt")
                        nc.sync.dma_start(
                            kts[:], src[r][:, :, lcc * 128:(lcc + 1) * 128]
                            .rearrange("kc p j -> p kc j"))
                        this_round = []
                        for e in range(lc // 2, 4):
                            ps_s = pp_s.tile([128, 256], F32, tag="s")
                            for kc in range(NCHK):
                                nc.tensor.matmul(ps_s[:], kts[:, kc, :],
                                                 qT[:, kc, e * 256:(e + 1) * 256],
                                                 start=(kc == 0), stop=(kc == NCHK - 1))
                            dst = aT[:, ABASE[e] + r * 2 * (e + 1) + lc, :]
                            if lc // 2 == e:
                                tmp = bs.tile([128, 256], BF16, tag="exps")
                                nc.scalar.activation(tmp[:], ps_s[:], AF.Exp, scale=SCALE)
                                mt = bm.tile([128, 256], BF16, tag="mask")
                                nc.sync.dma_start(mt[:], mask_d[e, r * 2 + lc % 2])
                                nc.vector.tensor_mul(dst, tmp[:], mt[:])
                            else:
                                nc.scalar.activation(dst, ps_s[:], AF.Exp, scale=SCALE)
                            # (r, lc) == (0, 0) is first for every class; class e
                            # ends at (1, 2e+1) in lc-major-then-r order.
                            this_round.append((e, (r, lc), dst))
                        for e, pos, src2 in pending:
                            nc.tensor.matmul(ps_sums[e][:], ones[:, 0:1], src2,
                                             start=(pos == (0, 0)),
                                             stop=(pos == (1, 2 * e + 1)))
                        pending = this_round
                    for e, pos, src2 in pending:
                        nc.tensor.matmul(ps_sums[e][:], ones[:, 0:1], src2,
                                         start=(pos == (0, 0)),
                                         stop=(pos == (1, 2 * e + 1)))
                    for e in range(4):
                        srow = bs.tile([1, 256], F32, tag="srow", name=f"srow{e}")
                        nc.scalar.copy(srow[:], ps_sums[e][:])
                        nc.scalar.dma_start(ssp_d[e], srow[:])
                    for e in range(4):
                        scol = bs.tile([128, 2], F32, tag="scol", name=f"scol{e}")
                        nc.sync.dma_start(scol[:],
                                          ssp_d[e].rearrange("(j p) -> p j", p=128))
                        nc.vector.reciprocal(recip[:, 2 * e:2 * e + 2], scol[:])

                # ---------- C: out^T = v^T-chunks @ attn^T ----------
                with (
                    nc.named_scope("attn_v"),
                    tc.tile_pool(name="cv", bufs=3) as cv,
                    tc.tile_pool(name="pp_o", bufs=2, space=bass.MemorySpace.PSUM) as pp_o,
                ):
                    for kvc in range(NCHK):
                        vts = cv.tile([128, NCHK, 128], BF16, tag="vt")
                        for r in range(2):
                            nc.sync.dma_start(
                                vts[:, r * 8:(r + 1) * 8, :],
                                vsp_d[r][:, :, kvc * 128:(kvc + 1) * 128]
                                .rearrange("lc p j -> p lc j"))
                        for e in range(4):
                            slots = [(r, lc) for r in range(2)
                                     for lc in range(2 * (e + 1))]
                            ps_o = pp_o.tile([128, 256], F32, tag="o")
                            for si, (r, lc) in enumerate(slots):
                                nc.tensor.matmul(
                                    ps_o[:], vts[:, r * 8 + lc, :],
                                    aT[:, ABASE[e] + r * 2 * (e + 1) + lc, :],
                                    start=(si == 0), stop=(si == len(slots) - 1))
                            if e % 2 == 0:
                                nc.scalar.copy(oT[:, kvc, e * 256:(e + 1) * 256],
                                               ps_o[:])
                            else:
                                nc.vector.tensor_copy(oT[:, kvc, e * 256:(e + 1) * 256],
                                                      ps_o[:])

                # ---------- D: y = diag(1/sums) (out @ Wo) + x ----------
                with (
                    nc.named_scope("o_proj"),
                    tc.tile_pool(name="dw", bufs=2) as dw,
                    tc.tile_pool(name="dx", bufs=4) as dx,
                    tc.tile_pool(name="dy", bufs=4) as dy,
                    tc.tile_pool(name="pp_y", bufs=6, space=bass.MemorySpace.PSUM) as pp_y,
                ):
                    for ht in range(4):
                        wo_s = dw.tile([128, NCHK, 512], BF16, tag="wo")
                        nc.sync.dma_start(wo_s[:],
                                          wo_d[:, ht * 512:(ht + 1) * 512]
                                          .rearrange("(kvc p) j -> p kvc j", p=128))
                        for qg in range(2):
                            psy = [pp_y.tile([128, 512], F32, tag="y", name=f"psy{i}")
                                   for i in range(4)]
                            for kvc in range(NCHK):
                                for i in range(4):
                                    qc = qg * 4 + i
                                    nc.tensor.matmul(psy[i][:],
                                                     oT[:, kvc, qc * 128:(qc + 1) * 128],
                                                     wo_s[:, kvc, :],
                                                     start=(kvc == 0),
                                                     stop=(kvc == NCHK - 1))
                            for i in range(4):
                                qc = qg * 4 + i
                                xres = dx.tile([128, 512], F32, tag="xr")
                                nc.sync.dma_start(xres[:],
                                                  x_d[PC[qc] * 128:(PC[qc] + 1) * 128,
                                                      ht * 512:(ht + 1) * 512])
                                ysb = dy.tile([128, 512], F32, tag="y")
                                nc.vector.scalar_tensor_tensor(
                                    ysb[:], psy[i][:], recip[:, qc:qc + 1], xres[:],
                                    OP.mult, OP.add)
                                nc.scalar.dma_start(y_d[qc * 128:(qc + 1) * 128,
                                                        ht * 512:(ht + 1) * 512],
                                                    ysb[:])
                if DBG:
                    nc.sync.dma_start(dbg_k[0], kspa_d[:])
                    nc.sync.dma_start(dbg_k[1], kspb_d[:])
                    nc.sync.dma_start(dbg_v[:], vsp_d[:])
                    nc.sync.dma_start(dbg_s[:], ssp_d[:])
                    for kc in range(NCHK):
                        nc.sync.dma_start(dbg_q[kc], qT[:, kc, :])
            p_bc0.__exit__(None, None, None)
    nc.compile()
    return nc


_NC_CACHE = None


def _get_nc():
    global _NC_CACHE
    if _NC_CACHE is None:
        _NC_CACHE = build()
    return _NC_CACHE


def make_in_maps(x, qkv, o_proj, gamma, beta):
    qkv = np.asarray(qkv)
    wq16 = np.ascontiguousarray(qkv[:, :KEY]).astype(ml_dtypes.bfloat16)
    wk16 = np.ascontiguousarray(qkv[:, KEY:2 * KEY]).astype(ml_dtypes.bfloat16)
    wv16 = np.ascontiguousarray(qkv[:, 2 * KEY:]).astype(ml_dtypes.bfloat16)
    wo16 = np.ascontiguousarray(o_proj).astype(ml_dtypes.bfloat16)
    gamma = np.ascontiguousarray(gamma, dtype=np.float32)
    beta = np.ascontiguousarray(beta, dtype=np.float32)
    in_maps, metas = [], []
    for c in range(8):
        b, h = c // 2, c % 2
        P = perm_chunks(h)
        ti = np.concatenate([np.arange(pc * 128, pc * 128 + 128) for pc in P])
        x_perm = np.ascontiguousarray(x[b][ti], dtype=np.float32)
        # mask[e][2r+j]: k tok-slot (rank r, quad e, j) holds true chunk
        # 4e+2r+j; q col c of class e is true row ti[512e+c].
        mask = np.zeros((4, 4, 128, 256), dtype=ml_dtypes.bfloat16)
        for e in range(4):
            qp = ti[512 * e:512 * e + 256]
            for r in range(2):
                for j in range(2):
                    kp = (4 * e + 2 * r + j) * 128 + np.arange(128)
                    mask[e, 2 * r + j] = (kp[:, None] <= qp[None, :]).astype(
                        ml_dtypes.bfloat16)
        in_maps.append({"x": x_perm, "wq": wq16, "wk": wk16, "wv": wv16,
                        "wo": wo16, "gamma": gamma, "beta": beta, "mask": mask})
        metas.append((b, ti))
    return in_maps, metas


def gather(results, metas, dtype):
    out = np.empty((B, S, H), dtype=dtype)
    qpos = np.concatenate([np.arange(512 * e, 512 * e + 256) for e in range(4)])
    for c, (b, ti) in enumerate(metas):
        out[b][ti[qpos]] = results[c]["y"]
    return out


def kernel(x, qkv, o_proj, gamma, beta, _trace=False):
    x = np.asarray(x, dtype=np.float32)
    nc = _get_nc()
    in_maps, metas = make_in_maps(x, qkv, o_proj, gamma, beta)
    res = run_bass_kernel_spmd(nc, in_maps, core_ids=list(range(8)), trace=_trace)
    out = gather(res.results, metas, np.float32)
    if _trace:
        kernel.last_result = res
    return out
